# revision 2
# baseline (speedup 1.0000x reference)
"""GCN layer (2x gcn_conv with GELU) on 8 Trainium2 NeuronCores — v3.

Contract: kernel(**inputs) takes the FULL inputs of reference.setup_inputs()
and returns the FULL [100000, 64] float32 output.

Strategy (graph/data parallel, sharded by destination node):
- Edges partitioned by dst across 8 cores (12500 dst nodes each).
- Per core, edges sorted by (dst-group of 128, src-window of 32768, src).
- Layer 1 "commuted": gather raw x rows (fp16) per edge via dma_gather;
  aggregate with one-hot fp16 S matrices (norm folded in) via TensorE into
  f32 PSUM, accumulated across gather calls per (batch, group).
- Dense transform (W1, GELU, W2) in f32 per 128-dst group; g2 = dinv * (z1@W2)
  converted to fp16.
- Exchange: instead of collectives, each core pushes its g2 chunks (7 chunks
  of <=2048 rows) to all 7 peers' SBUF staging via XOR-relative
  remote_dma_broadcast (1 real dest per instruction), then each receiver
  writes staging into its local DRAM g2 table (padded fp16 rows of 256 B).
- Layer 2: gather fp16 g2 rows per edge from the local table (per-core XOR
  layout), aggregate the same way, add self-loop, scale, bias, write out.

v3 changes vs v2 (measured ~25-30% faster on HW):
- dma_gather calls round-robin over 4 SWDGE queues (queue_num=ci%4): each
  queue's descriptor generation runs on a different GpSimd Q7 core pair,
  parallelizing the dominant per-token gather cost.
- Self-loop term dinv^2*x pre-scaled on host into xs (one ACT op/group saved).
- g2loc (f32) dropped; layer 2 reads the f16 g2send buffer directly (one ACT
  copy/group and 25KB/partition SBUF saved).
"""
import sys
sys.path.insert(0, "/opt/trn_rl_repo")

import numpy as np

N = 100000
FIN = 128
FOUT = 64
NC = 8
RS = N // NC            # 12500 dst rows per core
GSZ = 128               # dst group size
GP = (RS + GSZ - 1) // GSZ   # 98 groups per core (last has 84 nodes)
WIN = 32768             # src index window (int16 limit)
B = 6                   # dst groups per batch (PSUM banks: B agg + 2 transform)
CAP = 8192              # max tokens per dma_gather
AGL = 2048              # g2 rows per core per exchange chunk
KAG = (RS + AGL - 1) // AGL  # 7 chunks (last 212 rows)
GPC = AGL // GSZ        # dst groups per chunk (16)

# logical NC -> real NC on TRN2 (XOR-linear involution); relative rdma dests
# are XORed in real-NC space, so peer distance d maps to BASE[d].
BASEMAP = (0, 1, 2, 3, 6, 7, 4, 5)


def _set_config(**kw):
    """Override module constants (for scaled-down tests) and derived values."""
    g = globals()
    g.update(kw)
    g["RS"] = g["N"] // g["NC"]
    g["GP"] = (g["RS"] + g["GSZ"] - 1) // g["GSZ"]
    g["KAG"] = (g["RS"] + g["AGL"] - 1) // g["AGL"]
    g["GPC"] = g["AGL"] // g["GSZ"]
    assert g["AGL"] % g["GSZ"] == 0
    _cache.clear()

_cache = {}


# ----------------------------------------------------------------- host side

def _chunk_lens():
    return [min(AGL, RS - k * AGL) for k in range(KAG)]


def _flat_g2_row(src, c):
    """Node id -> row in core c's g2 table layout.

    Table layout per chunk k: [seg 0..NC) blocks of len_k rows, where seg
    i holds the shard of the core at XOR distance i in real-NC space:
    seg(cs) = BASE[cs] ^ BASE[c].
    """
    base = np.asarray(BASEMAP, dtype=np.int64)
    cs = src // RS
    r = src % RS
    k = r // AGL
    off = r - k * AGL
    len_k = np.minimum(RS - k * AGL, AGL)
    seg = base[cs] ^ base[c]
    return NC * AGL * k + seg * len_k + off


def _build_layer(rows_by_core, gl_by_core, dl_by_core, coef_by_core):
    """Shared program structure + per-core token data for one layer."""
    NW = (N + WIN - 1) // WIN
    has_coef = coef_by_core is not None
    per_core = []
    cell_cnt = np.zeros((NC, GP, NW), dtype=np.int64)
    for c in range(NC):
        rows, gl, dl = rows_by_core[c], gl_by_core[c], dl_by_core[c]
        win = rows // WIN
        order = np.lexsort((rows, win, gl))
        rows, gl, dl, win = rows[order], gl[order], dl[order], win[order]
        coef = coef_by_core[c][order] if has_coef else None
        np.add.at(cell_cnt[c], (gl, win), 1)
        flat_sizes = cell_cnt[c].reshape(-1)
        starts = np.concatenate([[0], np.cumsum(flat_sizes)[:-1]]).reshape(GP, NW)
        per_core.append(dict(rows=rows, dl=dl.astype(np.float32), coef=coef,
                             starts=starts))

    cell_max = cell_cnt.max(axis=0)
    cell_pad = ((cell_max + 15) // 16) * 16
    empty = cell_pad.sum(axis=1) == 0
    cell_pad[empty, 0] = 16

    batches = [list(range(b, min(b + B, GP))) for b in range(0, GP, B)]
    calls = []
    mdescs = []      # [call_i, col, g, m_index]
    total_cols_idx = 0

    def close_call(bi, w, gext, pos):
        nonlocal total_cols_idx
        ntok = ((pos + 127) // 128) * 128
        if ntok == 0:
            return
        ci = len(calls)
        calls.append(dict(batch=bi, w=w, ntok=ntok, gext=dict(gext),
                          idx_col0=total_cols_idx))
        total_cols_idx += ntok // 16
        for j in range(ntok // 128):
            lo, hi = j * 128, (j + 1) * 128
            for g, (s, e) in gext.items():
                if s < hi and e > lo:
                    mdescs.append([ci, j, g, len(mdescs)])

    for bi, groups in enumerate(batches):
        for w in range(NW):
            gext = {}
            pos = 0
            for g in groups:
                sz = int(cell_pad[g, w])
                if not sz:
                    continue
                assert sz <= CAP, f"cell {g},{w} = {sz} exceeds CAP"
                if pos + sz > CAP:
                    close_call(bi, w, gext, pos)
                    gext, pos = {}, 0
                gext[g] = (pos, pos + sz)
                pos += sz
            close_call(bi, w, gext, pos)

    # first/last call (and first/last mdesc within them) per (batch, group)
    first_call_of = {}
    last_call_of = {}
    for m in mdescs:
        ci, _, g, _ = m
        bkey = (calls[ci]["batch"], g)
        if bkey not in first_call_of:
            first_call_of[bkey] = (ci, m[3])
        last_call_of[bkey] = (ci, m[3])
    M = len(mdescs)

    idx_arr = np.zeros((NC, 16, total_cols_idx), dtype=np.int16)
    dst_arr = np.full((NC, M, 128), -1.0, dtype=np.float32)
    coef_arr = np.zeros((NC, M, 128), dtype=np.float32) if has_coef else None

    for c in range(NC):
        pc = per_core[c]
        for call in calls:
            w = call["w"]
            ntok = call["ntok"]
            stream_rows = np.zeros(ntok, dtype=np.int64)
            for g, (s, e) in call["gext"].items():
                cnt = int(cell_cnt[c, g, w])
                st = pc["starts"][g, w]
                if cnt:
                    stream_rows[s:s + cnt] = pc["rows"][st:st + cnt] - w * WIN
                    stream_rows[s + cnt:e] = stream_rows[s + cnt - 1]
            c0 = call["idx_col0"]
            idx_arr[c, :, c0:c0 + ntok // 16] = (
                stream_rows.astype(np.int16).reshape(-1, 16).T)
        for ci, j, g, mi in mdescs:
            call = calls[ci]
            w = call["w"]
            s, e = call["gext"][g]
            lo, hi = j * 128, (j + 1) * 128
            a = max(s, lo)
            cnt = int(cell_cnt[c, g, w])
            st = pc["starts"][g, w]
            real_hi = min(hi, s + cnt)
            if real_hi > a:
                k0, k1 = a - s, real_hi - s
                dst_arr[c, mi, a - lo:real_hi - lo] = pc["dl"][st + k0:st + k1]
                if has_coef:
                    coef_arr[c, mi, a - lo:real_hi - lo] = \
                        pc["coef"][st + k0:st + k1]

    out = dict(calls=calls, mdescs=mdescs, M=M,
               first_call_of=first_call_of, last_call_of=last_call_of,
               idx=np.tile(idx_arr, (1, 8, 1)),
               dst=np.ascontiguousarray(dst_arr.transpose(0, 2, 1)),
               total_idx_cols=total_cols_idx,
               batches=batches,
               max_ntok=max(c_["ntok"] for c_ in calls))
    if has_coef:
        out["coef"] = np.ascontiguousarray(coef_arr.transpose(0, 2, 1))
    return out


def _preprocess(x, edge_index, W1, b1, W2, b2):
    src = np.asarray(edge_index[0], dtype=np.int64)
    dst = np.asarray(edge_index[1], dtype=np.int64)
    deg = np.bincount(dst, minlength=N).astype(np.float32) + 1.0
    dinv = (1.0 / np.sqrt(deg)).astype(np.float32)

    core = dst // RS
    l1 = dict(rows=[], gl=[], dl=[], coef=[])
    l2 = dict(rows=[], gl=[], dl=[])
    for c in range(NC):
        m = core == c
        s, d = src[m], dst[m]
        rl = d - c * RS
        l1["rows"].append(s)
        l1["gl"].append(rl // GSZ)
        l1["dl"].append(rl % GSZ)
        l1["coef"].append((dinv[s] * dinv[d]).astype(np.float32))
        l2["rows"].append(_flat_g2_row(s, c))
        l2["gl"].append(rl // GSZ)
        l2["dl"].append(rl % GSZ)

    L1 = _build_layer(l1["rows"], l1["gl"], l1["dl"], l1["coef"])
    L2 = _build_layer(l2["rows"], l2["gl"], l2["dl"], None)

    dinv_pc = np.zeros((NC, 128, GP), dtype=np.float32)
    xs_pc = np.zeros((NC, GP * GSZ, FIN), dtype=np.float32)
    for c in range(NC):
        dvp = np.zeros(GP * GSZ, dtype=np.float32)
        dvp[:RS] = dinv[c * RS:(c + 1) * RS]
        dinv_pc[c] = dvp.reshape(GP, GSZ).T
        # self-loop term pre-scaled by dinv^2 (saves one ACT op per group)
        xs_pc[c, :RS] = x[c * RS:(c + 1) * RS] *             (dinv[c * RS:(c + 1) * RS] ** 2)[:, None]

    b1bc = np.tile(np.asarray(b1, np.float32)[None, :], (128, 1))
    b2bc = np.tile(np.asarray(b2, np.float32)[None, :], (128, 1))
    return dict(L1=L1, L2=L2, dinv_pc=dinv_pc, xs_pc=xs_pc,
                b1bc=b1bc, b2bc=b2bc,
                xh=np.ascontiguousarray(np.asarray(x, np.float16)),
                W1=np.ascontiguousarray(np.asarray(W1, np.float32)),
                W2=np.ascontiguousarray(np.asarray(W2, np.float32)))


def _make_in_maps(pp, x_unused=None):
    in_maps = []
    for c in range(NC):
        in_maps.append({
            "xh": pp["xh"],
            "xs": pp["xs_pc"][c],
            "idx1": pp["L1"]["idx"][c],
            "idx2": pp["L2"]["idx"][c],
            "dst1": pp["L1"]["dst"][c],
            "coef1": pp["L1"]["coef"][c],
            "dst2": pp["L2"]["dst"][c],
            "dinv_pc": pp["dinv_pc"][c],
            "W1": pp["W1"], "W2": pp["W2"],
            "b1bc": pp["b1bc"], "b2bc": pp["b2bc"],
        })
    return in_maps


# --------------------------------------------------------------- device side

def _build_nc(pp, act="gelu"):
    import concourse.bacc as bacc
    import concourse.tile as tile
    from concourse import mybir
    from concourse.masks import make_identity

    L1, L2 = pp["L1"], pp["L2"]
    nc = bacc.Bacc(num_devices=NC, num_swdge_queues=4)
    f32 = mybir.dt.float32
    f16 = mybir.dt.float16

    NW = (N + WIN - 1) // WIN
    winlen = [min(WIN, N - w * WIN) for w in range(NW)]
    lens = _chunk_lens()
    NTAB = NC * RS  # g2 table rows (padded fp16 rows of 128)
    # chunk k of the table must sit inside one gather window
    for k in range(KAG):
        assert (NC * AGL * k) // WIN == (NC * AGL * k + NC * lens[k] - 1) // WIN

    t_xh = nc.dram_tensor("xh", [N, FIN], f16, kind="ExternalInput")
    t_xs = nc.dram_tensor("xs", [GP * GSZ, FIN], f32, kind="ExternalInput")
    t_idx1 = nc.dram_tensor("idx1", [128, L1["total_idx_cols"]],
                            mybir.dt.int16, kind="ExternalInput")
    t_idx2 = nc.dram_tensor("idx2", [128, L2["total_idx_cols"]],
                            mybir.dt.int16, kind="ExternalInput")
    t_dst1 = nc.dram_tensor("dst1", [128, L1["M"]], f32, kind="ExternalInput")
    t_coef1 = nc.dram_tensor("coef1", [128, L1["M"]], f32,
                             kind="ExternalInput")
    t_dst2 = nc.dram_tensor("dst2", [128, L2["M"]], f32, kind="ExternalInput")
    t_dinv = nc.dram_tensor("dinv_pc", [128, GP], f32, kind="ExternalInput")
    t_w1 = nc.dram_tensor("W1", [FIN, FOUT], f32, kind="ExternalInput")
    t_w2 = nc.dram_tensor("W2", [FOUT, FOUT], f32, kind="ExternalInput")
    t_b1 = nc.dram_tensor("b1bc", [128, FOUT], f32, kind="ExternalInput")
    t_b2 = nc.dram_tensor("b2bc", [128, FOUT], f32, kind="ExternalInput")
    t_out = nc.dram_tensor("out", [RS, FOUT], f32, kind="ExternalOutput")
    t_g2t = nc.dram_tensor("g2t", [NTAB, FIN], f16, kind="Internal")

    actf = {"gelu": mybir.ActivationFunctionType.Gelu,
            "tanh": mybir.ActivationFunctionType.Tanh}[act]

    rsems = [nc.alloc_semaphore(f"g2arr{k}") for k in range(KAG)]
    lsem = nc.alloc_semaphore("g2sent")
    dsem = nc.alloc_semaphore("g2wr")

    # SBUF state shared across the two tile contexts (raw, not pool-managed)
    g2send = nc.alloc_sbuf_tensor("g2send", [128, GP, FOUT], f16)
    stage = nc.alloc_sbuf_tensor("stage", [128, KAG, NC - 1, GPC, FOUT], f16)
    iota = nc.alloc_sbuf_tensor("iotah", [128, 128], f16)
    dinv_t = nc.alloc_sbuf_tensor("dinvt", [128, GP], f32)
    b2_t = nc.alloc_sbuf_tensor("b2t", [128, FOUT], f32)

    def run_phase(tc, L, t_idx, dst_t, coef_t, elem, src_spaces, post_fn,
                  tag, gelem=None):
        from concourse import mybir
        gelem = elem if gelem is None else gelem
        with (
            tc.tile_pool(name=f"gat{tag}", bufs=2) as gp_,
            tc.tile_pool(name=f"idx{tag}", bufs=4) as ip_,
            tc.tile_pool(name=f"agg{tag}", bufs=B, space="PSUM") as ap_,
            tc.tile_pool(name=f"post{tag}", bufs=4) as wp_,
            tc.tile_pool(name=f"sgen{tag}", bufs=8) as sp_,
            tc.tile_pool(name=f"pp{tag}", bufs=2, space="PSUM") as pp_,
        ):
            mi_by_call = {}
            for m in L["mdescs"]:
                mi_by_call.setdefault(m[0], []).append(m)
            psum_of = {}
            cur_batch = [-1]

            for ci, call in enumerate(L["calls"]):
                ntok = call["ntok"]
                ncols = ntok // 128
                bi = call["batch"]
                groups_b = L["batches"][bi]
                if bi != cur_batch[0]:
                    cur_batch[0] = bi
                    psum_of.clear()
                    for g in groups_b:
                        t = ap_.tile([128, elem], mybir.dt.float32,
                                     tag=f"a{tag}", name=f"aggp{tag}",
                                     space="PSUM")
                        psum_of[g] = t[:, :]
                gtile = gp_.tile([128, L["max_ntok"] // 128, gelem],
                                 mybir.dt.float16, tag=f"g{tag}")
                idxt = ip_.tile([128, L["max_ntok"] // 16],
                                mybir.dt.int16, tag=f"i{tag}")
                c0 = call["idx_col0"]
                nc.sync.dma_start(idxt[:, :ntok // 16],
                                  t_idx[:, c0:c0 + ntok // 16])
                nc.gpsimd.dma_gather(
                    out_ap=gtile[:, :ncols, :],
                    in_ap=src_spaces[call["w"]],
                    idxs_ap=idxt[:, :ntok // 16],
                    num_idxs=ntok,
                    num_idxs_reg=ntok,
                    elem_size=gelem,
                    single_packet=False,
                    queue_num=ci % 4,
                )
                for _, j, g, mi in mi_by_call.get(ci, []):
                    bkey = (bi, g)
                    st = L["first_call_of"][bkey] == (ci, mi)
                    sp = L["last_call_of"][bkey] == (ci, mi)
                    S = sp_.tile([128, 128], f16, tag=f"S{tag}")
                    if coef_t is not None:
                        nc.vector.tensor_scalar(
                            out=S[:], in0=iota[:, :],
                            scalar1=dst_t[:, mi:mi + 1],
                            scalar2=coef_t[:, mi:mi + 1],
                            op0=mybir.AluOpType.is_equal,
                            op1=mybir.AluOpType.mult)
                    else:
                        nc.vector.tensor_scalar(
                            out=S[:], in0=iota[:, :],
                            scalar1=dst_t[:, mi:mi + 1],
                            scalar2=None,
                            op0=mybir.AluOpType.is_equal)
                    nc.tensor.matmul(psum_of[g], lhsT=S[:],
                                     rhs=gtile[:, j, :elem],
                                     start=st, stop=sp)
                    if sp:
                        post_fn(g, psum_of[g], wp_, pp_)

    # ================= context A: layer 1 + transform + exchange sends
    with tile.TileContext(nc) as tc:
        with (
            tc.tile_pool(name="const", bufs=1) as cp,
        ):
            ident = cp.tile([128, 128], f32)
            make_identity(nc, ident[:])
            iota_i = cp.tile([128, 128], mybir.dt.int32)
            nc.gpsimd.iota(iota_i[:], pattern=[[1, 128]], base=0,
                           channel_multiplier=0)
            nc.vector.tensor_copy(iota[:, :], iota_i[:])
            w1_t = cp.tile([FIN, FOUT], f32)
            w2_t = cp.tile([FOUT, FOUT], f32)
            b1_t = cp.tile([128, FOUT], f32)
            for tt, src_t in ((w1_t, t_w1), (w2_t, t_w2), (b1_t, t_b1)):
                nc.sync.dma_start(tt[:], src_t[:, :])
            nc.sync.dma_start(dinv_t[:, :], t_dinv[:, :])
            nc.sync.dma_start(b2_t[:, :], t_b2[:, :])
            dst1_t = cp.tile([128, L1["M"]], f32)
            coef1_t = cp.tile([128, L1["M"]], f32)
            nc.sync.dma_start(dst1_t[:], t_dst1[:, :])
            nc.sync.dma_start(coef1_t[:], t_coef1[:, :])

            def send_chunk(k):
                gcnt = (lens[k] + GSZ - 1) // GSZ
                g0 = k * GPC
                src_ap = g2send[:, g0:g0 + gcnt, :]
                for i in range(1, NC):
                    rd = [None] * 8
                    rd[i] = (0, i)
                    nc.gpsimd.remote_dma_broadcast(
                        out_ap=stage[:, k, i - 1, 0:gcnt, :],
                        in_ap=src_ap,
                        remote_sem=rsems[k], local_sem=lsem,
                        rdests=rd)
                nc.gpsimd.trigger_dma(count=None)
                # own shard -> local table, overlapped with layer 1 (needs
                # no remote wait; completion counted into dsem)
                base = NC * AGL * k
                jf = lens[k] // GSZ
                rem = lens[k] - jf * GSZ
                if jf:
                    dap = t_g2t[base:base + jf * GSZ, 0:FOUT].rearrange(
                        "(j p) c -> p j c", p=GSZ)
                    nc.sync.dma_start(dap, src_ap[:, :jf, :])
                if rem:
                    nc.sync.dma_start(
                        t_g2t[base + jf * GSZ:base + lens[k], 0:FOUT],
                        src_ap[:rem, jf, :])

            def post_l1(g, agg, wp_, pp_):
                xd = wp_.tile([128, FIN], f32, tag="xd")
                nc.sync.dma_start(xd[:], t_xs[g * GSZ:(g + 1) * GSZ, :])
                v = wp_.tile([128, FIN], f32, tag="v")
                nc.vector.tensor_tensor(out=v[:], in0=xd[:], in1=agg,
                                        op=mybir.AluOpType.add)
                bank = pp_.tile([128, 512], f32, tag="pb", space="PSUM")
                vT_p = bank[:, 0:128]
                h1_p = bank[:, 128:128 + FOUT]
                z1T_p = bank[:64, 192:320]
                h2_p = bank[:, 320:320 + FOUT]
                nc.tensor.transpose(vT_p, v[:], ident[:])
                vT = wp_.tile([128, 128], f32, tag="vTs")
                nc.scalar.activation(vT[:], vT_p,
                                     mybir.ActivationFunctionType.Copy)
                nc.tensor.matmul(h1_p, lhsT=vT[:], rhs=w1_t[:], start=True,
                                 stop=True)
                h1b = wp_.tile([128, FOUT], f32, tag="h1b")
                nc.vector.tensor_tensor(out=h1b[:], in0=h1_p, in1=b1_t[:],
                                        op=mybir.AluOpType.add)
                z1 = wp_.tile([128, FOUT], f32, tag="z1")
                nc.scalar.activation(z1[:], h1b[:], actf)
                nc.tensor.transpose(z1T_p, z1[:], ident[:])
                z1T = wp_.tile([FOUT, 128], f32, tag="z1Ts")
                nc.scalar.activation(z1T[:], z1T_p,
                                     mybir.ActivationFunctionType.Copy)
                nc.tensor.matmul(h2_p, lhsT=z1T[:], rhs=w2_t[:], start=True,
                                 stop=True)
                nc.scalar.activation(g2send[:, g, :], h2_p,
                                     mybir.ActivationFunctionType.Copy,
                                     scale=dinv_t[:, g:g + 1])
                k = g // GPC
                if g == min(GP, (k + 1) * GPC) - 1:
                    send_chunk(k)

            run_phase(tc, L1, t_idx1, dst1_t, coef1_t, FIN,
                      [t_xh[w * WIN:w * WIN + winlen[w], :]
                       for w in range(NW)],
                      post_l1, "1")

    # ================= raw middle: wait for remote chunks, build g2 table
    # Peer segments only (self shards were written during layer 1). Issued
    # from the Activation engine so SP is free to prefetch layer-2 idx/const
    # tiles during this span.
    ndma = 0
    for k in range(KAG):
        nc.scalar.wait_ge(rsems[k], (NC - 1) * 2)
        gcnt = (lens[k] + GSZ - 1) // GSZ
        jf = lens[k] // GSZ          # full 128-row groups
        rem = lens[k] - jf * GSZ
        base = NC * AGL * k
        if jf == gcnt:
            dap = t_g2t[base + lens[k]:base + NC * lens[k], 0:FOUT].rearrange(
                "(s j p) c -> p (s j) c", p=GSZ, s=NC - 1).opt()
            nc.scalar.dma_start(dap, stage[:, k, :, 0:gcnt, :].opt()) \
                .then_inc(dsem, 16)
            ndma += 1
            continue
        for i in range(1, NC):
            sap = stage[:, k, i - 1, 0:gcnt, :]
            base_i = base + i * lens[k]
            if jf:
                dap = t_g2t[base_i:base_i + jf * GSZ, 0:FOUT].rearrange(
                    "(j p) c -> p j c", p=GSZ)
                nc.scalar.dma_start(dap, sap[:, :jf, :]).then_inc(dsem, 16)
                ndma += 1
            if rem:
                nc.scalar.dma_start(
                    t_g2t[base_i + jf * GSZ:base_i + lens[k], 0:FOUT],
                    sap[:rem, jf, :]).then_inc(dsem, 16)
                ndma += 1
    nc.gpsimd.wait_ge(dsem, 16 * ndma)

    # ================= context B: layer 2
    tablen = [min(WIN, NTAB - w * WIN) for w in range(NW)]
    with tile.TileContext(nc) as tc:
        with tc.tile_pool(name="constB", bufs=1) as cpb:
            dst2_t = cpb.tile([128, L2["M"]], f32)
            nc.sync.dma_start(dst2_t[:], t_dst2[:, :])

            def post_l2(g, agg, wp_, pp_):
                t1 = wp_.tile([128, FOUT], f32, tag="t1")
                nc.vector.tensor_tensor(out=t1[:], in0=agg,
                                        in1=g2send[:, g, :],
                                        op=mybir.AluOpType.add)
                t2 = wp_.tile([128, FOUT], f32, tag="t2")
                nc.scalar.activation(t2[:], t1[:],
                                     mybir.ActivationFunctionType.Copy,
                                     scale=dinv_t[:, g:g + 1])
                t3 = wp_.tile([128, FOUT], f32, tag="t3")
                nc.vector.tensor_tensor(out=t3[:], in0=t2[:], in1=b2_t[:, :],
                                        op=mybir.AluOpType.add)
                nrow = min(GSZ, RS - g * GSZ)
                nc.sync.dma_start(t_out[g * GSZ:g * GSZ + nrow, :],
                                  t3[:nrow, :])

            run_phase(tc, L2, t_idx2, dst2_t, None, FOUT,
                      [t_g2t[w * WIN:w * WIN + tablen[w], :]
                       for w in range(NW)],
                      post_l2, "2", gelem=FIN)

    nc.compile()
    return nc


def _run(inputs, act="gelu", trace=False, use_sim=False, trace_kwargs=None):
    x = np.ascontiguousarray(np.asarray(inputs["x"], np.float32))
    key = (hash(np.asarray(inputs["edge_index"]).tobytes()), act)
    if key not in _cache:
        pp = _preprocess(x, np.asarray(inputs["edge_index"]),
                         inputs["W1"], inputs["b1"], inputs["W2"],
                         inputs["b2"])
        nc = _build_nc(pp, act=act)
        _cache.clear()
        _cache[key] = (pp, nc)
    pp, nc = _cache[key]

    in_maps = _make_in_maps(pp)
    if use_sim:
        from concourse.bass_interp import MultiCoreSim
        sim = MultiCoreSim(nc, num_cores=NC, require_finite=False, require_nnan=False)
        for ci, core in sim.cores.items():
            for k, v in in_maps[ci].items():
                core.tensor(k)[:] = v
        sim.simulate()
        outs = [np.array(core.tensor("out"))
                for _, core in sorted(sim.cores.items())]
        return np.concatenate(outs, 0), None
    from concourse.bass_utils import run_bass_kernel_spmd
    res = run_bass_kernel_spmd(nc, in_maps, core_ids=list(range(NC)),
                               trace=trace, **(trace_kwargs or {}))
    out = np.concatenate([res.results[c]["out"] for c in range(NC)], 0)
    return out, res


def kernel(**inputs) -> np.ndarray:
    out, _ = _run(inputs)
    return out


def bench(inputs, act="gelu", iters=8):
    """Measure per-execution device time by chaining `iters` executions of
    the NEFF inside one jit (outputs feed the next iteration's output
    operands, defeating CSE) and comparing against a 1-iteration call."""
    import time
    import jax
    from jax.sharding import Mesh, PartitionSpec
    from jax.experimental.shard_map import shard_map
    from concourse import bass2jax as b2j

    key = (hash(np.asarray(inputs["edge_index"]).tobytes()), act)
    if key not in _cache:
        _run(inputs, act=act)   # build + correctness path
    pp, nc = _cache[key]
    b2j.install_neuronx_cc_hook()

    in_maps = _make_in_maps(pp)

    in_names, out_names, out_avals, zero_outs = [], [], [], []
    import concourse.mybir as mb
    pid_name = (nc.partition_id_tensor.name
                if nc.partition_id_tensor is not None else None)
    for alloc in nc.m.functions[0].allocations:
        if not isinstance(alloc, mb.MemoryLocationSet):
            continue
        name = alloc.memorylocations[0].name
        if alloc.kind == "ExternalInput":
            if name == pid_name:
                continue
            in_names.append(name)
        elif alloc.kind == "ExternalOutput":
            out_names.append(name)
            shape = tuple(alloc.tensor_shape)
            dtype = mb.dt.np(alloc.dtype)
            out_avals.append(jax.core.ShapedArray(shape, dtype))
            zero_outs.append(np.zeros(shape, dtype))
    n_params = len(in_names)
    all_names = in_names + out_names
    if pid_name is not None:
        all_names = all_names + [pid_name]

    def one_call(params, outs_in):
        extra = ([b2j.partition_id_tensor()] if pid_name is not None else [])
        outs = b2j._bass_exec_p.bind(
            *params, *outs_in, *extra,
            out_avals=tuple(out_avals),
            in_names=tuple(all_names),
            out_names=tuple(out_names),
            lowering_input_output_aliases=(),
            sim_require_finite=True,
            sim_require_nnan=True,
            nc=nc,
        )
        return list(outs)

    def _body(*args):
        params = list(args[:n_params])
        outs = list(args[n_params:])
        outs = one_call(params, outs)
        return tuple(outs)

    devices = jax.devices()[:NC]
    mesh = Mesh(np.asarray(devices), ("core",))
    specs = (PartitionSpec("core"),)
    per_core = [[np.asarray(m[nm]) for nm in in_names] for m in in_maps]
    concat_in = [np.concatenate([per_core[c][i] for c in range(NC)], 0)
                 for i in range(n_params)]
    concat_zeros = [np.zeros((NC * z.shape[0], *z.shape[1:]), z.dtype)
                    for z in zero_outs]

    nin = n_params + len(out_names)
    fn = jax.jit(shard_map(_body, mesh=mesh,
                           in_specs=specs * nin,
                           out_specs=specs * len(out_names),
                           check_rep=False),
                 donate_argnums=tuple(range(n_params, nin)))
    from jax.sharding import NamedSharding
    shard = NamedSharding(mesh, PartitionSpec("core"))
    dev_in = [jax.device_put(a, shard) for a in concat_in]
    outs = [jax.device_put(a, shard) for a in concat_zeros]
    outs = fn(*dev_in, *outs)          # warm: compile + first exec
    jax.block_until_ready(outs)

    results = {}
    for k in (1, iters):
        best = None
        for _ in range(3):
            t0 = time.perf_counter()
            o = outs
            for _ in range(k):
                o = fn(*dev_in, *o)
            jax.block_until_ready(o)
            dt = time.perf_counter() - t0
            outs = o
            best = dt if best is None else min(best, dt)
        results[k] = best
    per_iter_ns = (results[iters] - results[1]) / (iters - 1) * 1e9
    return per_iter_ns, results



# revision 3
# speedup vs baseline: 1.3096x; 1.3096x over previous
"""GCN layer (2x gcn_conv with GELU) on 8 Trainium2 NeuronCores — v2.

Contract: kernel(**inputs) takes the FULL inputs of reference.setup_inputs()
and returns the FULL [100000, 64] float32 output.

Strategy (graph/data parallel, sharded by destination node):
- Edges partitioned by dst across 8 cores (12500 dst nodes each).
- Per core, edges sorted by (dst-group of 128, src-window of 32768, src).
- Layer 1 "commuted": gather raw x rows (fp16) per edge via dma_gather;
  aggregate with one-hot fp16 S matrices (norm folded in) via TensorE into
  f32 PSUM, accumulated across gather calls per (batch, group).
- Dense transform (W1, GELU, W2) in f32 per 128-dst group; g2 = dinv * (z1@W2)
  converted to fp16.
- Exchange: instead of collectives, each core pushes its g2 chunks (7 chunks
  of <=2048 rows) to all 7 peers' SBUF staging via XOR-relative
  remote_dma_broadcast (1 real dest per instruction), then each receiver
  writes staging into its local DRAM g2 table (padded fp16 rows of 256 B).
- Layer 2: gather fp16 g2 rows per edge from the local table (per-core XOR
  layout), aggregate the same way, add self-loop, scale, bias, write out.
"""
import sys
sys.path.insert(0, "/opt/trn_rl_repo")

import numpy as np

N = 100000
FIN = 128
FOUT = 64
NC = 8
RS = N // NC            # 12500 dst rows per core
GSZ = 128               # dst group size
GP = (RS + GSZ - 1) // GSZ   # 98 groups per core (last has 84 nodes)
WIN = 32768             # src index window (int16 limit)
B = 6                   # dst groups per batch (PSUM banks: B agg + 2 transform)
CAP = 8192              # max tokens per dma_gather
AGL = 2048              # g2 rows per core per exchange chunk
KAG = (RS + AGL - 1) // AGL  # 7 chunks (last 212 rows)
GPC = AGL // GSZ        # dst groups per chunk (16)

# logical NC -> real NC on TRN2 (XOR-linear involution); relative rdma dests
# are XORed in real-NC space, so peer distance d maps to BASE[d].
BASEMAP = (0, 1, 2, 3, 6, 7, 4, 5)


def _set_config(**kw):
    """Override module constants (for scaled-down tests) and derived values."""
    g = globals()
    g.update(kw)
    g["RS"] = g["N"] // g["NC"]
    g["GP"] = (g["RS"] + g["GSZ"] - 1) // g["GSZ"]
    g["KAG"] = (g["RS"] + g["AGL"] - 1) // g["AGL"]
    g["GPC"] = g["AGL"] // g["GSZ"]
    assert g["AGL"] % g["GSZ"] == 0
    _cache.clear()

_cache = {}


# ----------------------------------------------------------------- host side

def _chunk_lens():
    return [min(AGL, RS - k * AGL) for k in range(KAG)]


def _flat_g2_row(src, c):
    """Node id -> row in core c's g2 table layout.

    Table layout per chunk k: [seg 0..NC) blocks of len_k rows, where seg
    i holds the shard of the core at XOR distance i in real-NC space:
    seg(cs) = BASE[cs] ^ BASE[c].
    """
    base = np.asarray(BASEMAP, dtype=np.int64)
    cs = src // RS
    r = src % RS
    k = r // AGL
    off = r - k * AGL
    len_k = np.minimum(RS - k * AGL, AGL)
    seg = base[cs] ^ base[c]
    return NC * AGL * k + seg * len_k + off


def _build_layer(rows_by_core, gl_by_core, dl_by_core, coef_by_core):
    """Shared program structure + per-core token data for one layer."""
    NW = (N + WIN - 1) // WIN
    has_coef = coef_by_core is not None
    per_core = []
    cell_cnt = np.zeros((NC, GP, NW), dtype=np.int64)
    for c in range(NC):
        rows, gl, dl = rows_by_core[c], gl_by_core[c], dl_by_core[c]
        win = rows // WIN
        order = np.lexsort((rows, win, gl))
        rows, gl, dl, win = rows[order], gl[order], dl[order], win[order]
        coef = coef_by_core[c][order] if has_coef else None
        np.add.at(cell_cnt[c], (gl, win), 1)
        flat_sizes = cell_cnt[c].reshape(-1)
        starts = np.concatenate([[0], np.cumsum(flat_sizes)[:-1]]).reshape(GP, NW)
        per_core.append(dict(rows=rows, dl=dl.astype(np.float32), coef=coef,
                             starts=starts))

    cell_max = cell_cnt.max(axis=0)
    cell_pad = ((cell_max + 15) // 16) * 16
    empty = cell_pad.sum(axis=1) == 0
    cell_pad[empty, 0] = 16

    batches = [list(range(b, min(b + B, GP))) for b in range(0, GP, B)]
    calls = []
    mdescs = []      # [call_i, col, g, m_index]
    total_cols_idx = 0

    def close_call(bi, w, gext, pos):
        nonlocal total_cols_idx
        ntok = ((pos + 127) // 128) * 128
        if ntok == 0:
            return
        ci = len(calls)
        calls.append(dict(batch=bi, w=w, ntok=ntok, gext=dict(gext),
                          idx_col0=total_cols_idx))
        total_cols_idx += ntok // 16
        for j in range(ntok // 128):
            lo, hi = j * 128, (j + 1) * 128
            for g, (s, e) in gext.items():
                if s < hi and e > lo:
                    mdescs.append([ci, j, g, len(mdescs)])

    for bi, groups in enumerate(batches):
        for w in range(NW):
            gext = {}
            pos = 0
            for g in groups:
                sz = int(cell_pad[g, w])
                if not sz:
                    continue
                assert sz <= CAP, f"cell {g},{w} = {sz} exceeds CAP"
                if pos + sz > CAP:
                    close_call(bi, w, gext, pos)
                    gext, pos = {}, 0
                gext[g] = (pos, pos + sz)
                pos += sz
            close_call(bi, w, gext, pos)

    # first/last call (and first/last mdesc within them) per (batch, group)
    first_call_of = {}
    last_call_of = {}
    for m in mdescs:
        ci, _, g, _ = m
        bkey = (calls[ci]["batch"], g)
        if bkey not in first_call_of:
            first_call_of[bkey] = (ci, m[3])
        last_call_of[bkey] = (ci, m[3])
    M = len(mdescs)

    idx_arr = np.zeros((NC, 16, total_cols_idx), dtype=np.int16)
    dst_arr = np.full((NC, M, 128), -1.0, dtype=np.float32)
    coef_arr = np.zeros((NC, M, 128), dtype=np.float32) if has_coef else None

    for c in range(NC):
        pc = per_core[c]
        for call in calls:
            w = call["w"]
            ntok = call["ntok"]
            stream_rows = np.zeros(ntok, dtype=np.int64)
            for g, (s, e) in call["gext"].items():
                cnt = int(cell_cnt[c, g, w])
                st = pc["starts"][g, w]
                if cnt:
                    stream_rows[s:s + cnt] = pc["rows"][st:st + cnt] - w * WIN
                    stream_rows[s + cnt:e] = stream_rows[s + cnt - 1]
            c0 = call["idx_col0"]
            idx_arr[c, :, c0:c0 + ntok // 16] = (
                stream_rows.astype(np.int16).reshape(-1, 16).T)
        for ci, j, g, mi in mdescs:
            call = calls[ci]
            w = call["w"]
            s, e = call["gext"][g]
            lo, hi = j * 128, (j + 1) * 128
            a = max(s, lo)
            cnt = int(cell_cnt[c, g, w])
            st = pc["starts"][g, w]
            real_hi = min(hi, s + cnt)
            if real_hi > a:
                k0, k1 = a - s, real_hi - s
                dst_arr[c, mi, a - lo:real_hi - lo] = pc["dl"][st + k0:st + k1]
                if has_coef:
                    coef_arr[c, mi, a - lo:real_hi - lo] = \
                        pc["coef"][st + k0:st + k1]

    out = dict(calls=calls, mdescs=mdescs, M=M,
               first_call_of=first_call_of, last_call_of=last_call_of,
               idx=np.tile(idx_arr, (1, 8, 1)),
               dst=np.ascontiguousarray(dst_arr.transpose(0, 2, 1)),
               total_idx_cols=total_cols_idx,
               batches=batches,
               max_ntok=max(c_["ntok"] for c_ in calls))
    if has_coef:
        out["coef"] = np.ascontiguousarray(coef_arr.transpose(0, 2, 1))
    return out


def _preprocess(x, edge_index, W1, b1, W2, b2):
    src = np.asarray(edge_index[0], dtype=np.int64)
    dst = np.asarray(edge_index[1], dtype=np.int64)
    deg = np.bincount(dst, minlength=N).astype(np.float32) + 1.0
    dinv = (1.0 / np.sqrt(deg)).astype(np.float32)

    core = dst // RS
    l1 = dict(rows=[], gl=[], dl=[], coef=[])
    l2 = dict(rows=[], gl=[], dl=[])
    for c in range(NC):
        m = core == c
        s, d = src[m], dst[m]
        rl = d - c * RS
        l1["rows"].append(s)
        l1["gl"].append(rl // GSZ)
        l1["dl"].append(rl % GSZ)
        l1["coef"].append((dinv[s] * dinv[d]).astype(np.float32))
        l2["rows"].append(_flat_g2_row(s, c))
        l2["gl"].append(rl // GSZ)
        l2["dl"].append(rl % GSZ)

    L1 = _build_layer(l1["rows"], l1["gl"], l1["dl"], l1["coef"])
    L2 = _build_layer(l2["rows"], l2["gl"], l2["dl"], None)

    dinv_pc = np.zeros((NC, 128, GP), dtype=np.float32)
    xs_pc = np.zeros((NC, GP * GSZ, FIN), dtype=np.float32)
    for c in range(NC):
        dvp = np.zeros(GP * GSZ, dtype=np.float32)
        dvp[:RS] = dinv[c * RS:(c + 1) * RS]
        dinv_pc[c] = dvp.reshape(GP, GSZ).T
        # self-loop term pre-scaled by dinv^2 (saves one ACT op per group)
        xs_pc[c, :RS] = x[c * RS:(c + 1) * RS] *             (dinv[c * RS:(c + 1) * RS] ** 2)[:, None]

    b1bc = np.tile(np.asarray(b1, np.float32)[None, :], (128, 1))
    b2bc = np.tile(np.asarray(b2, np.float32)[None, :], (128, 1))
    return dict(L1=L1, L2=L2, dinv_pc=dinv_pc, xs_pc=xs_pc,
                b1bc=b1bc, b2bc=b2bc,
                xh=np.ascontiguousarray(np.asarray(x, np.float16)),
                W1=np.ascontiguousarray(np.asarray(W1, np.float32)),
                W2=np.ascontiguousarray(np.asarray(W2, np.float32)))


def _make_in_maps(pp, x_unused=None):
    in_maps = []
    for c in range(NC):
        in_maps.append({
            "xh": pp["xh"],
            "xs": pp["xs_pc"][c],
            "idx1": pp["L1"]["idx"][c],
            "idx2": pp["L2"]["idx"][c],
            "dst1": pp["L1"]["dst"][c],
            "coef1": pp["L1"]["coef"][c],
            "dst2": pp["L2"]["dst"][c],
            "dinv_pc": pp["dinv_pc"][c],
            "W1": pp["W1"], "W2": pp["W2"],
            "b1bc": pp["b1bc"], "b2bc": pp["b2bc"],
        })
    return in_maps


# --------------------------------------------------------------- device side

def _build_nc(pp, act="gelu"):
    import concourse.bacc as bacc
    import concourse.tile as tile
    from concourse import mybir
    from concourse.masks import make_identity

    L1, L2 = pp["L1"], pp["L2"]
    nc = bacc.Bacc(num_devices=NC, num_swdge_queues=4)
    f32 = mybir.dt.float32
    f16 = mybir.dt.float16

    NW = (N + WIN - 1) // WIN
    winlen = [min(WIN, N - w * WIN) for w in range(NW)]
    lens = _chunk_lens()
    NTAB = NC * RS  # g2 table rows (padded fp16 rows of 128)
    # chunk k of the table must sit inside one gather window
    for k in range(KAG):
        assert (NC * AGL * k) // WIN == (NC * AGL * k + NC * lens[k] - 1) // WIN

    t_xh = nc.dram_tensor("xh", [N, FIN], f16, kind="ExternalInput")
    t_xs = nc.dram_tensor("xs", [GP * GSZ, FIN], f32, kind="ExternalInput")
    t_idx1 = nc.dram_tensor("idx1", [128, L1["total_idx_cols"]],
                            mybir.dt.int16, kind="ExternalInput")
    t_idx2 = nc.dram_tensor("idx2", [128, L2["total_idx_cols"]],
                            mybir.dt.int16, kind="ExternalInput")
    t_dst1 = nc.dram_tensor("dst1", [128, L1["M"]], f32, kind="ExternalInput")
    t_coef1 = nc.dram_tensor("coef1", [128, L1["M"]], f32,
                             kind="ExternalInput")
    t_dst2 = nc.dram_tensor("dst2", [128, L2["M"]], f32, kind="ExternalInput")
    t_dinv = nc.dram_tensor("dinv_pc", [128, GP], f32, kind="ExternalInput")
    t_w1 = nc.dram_tensor("W1", [FIN, FOUT], f32, kind="ExternalInput")
    t_w2 = nc.dram_tensor("W2", [FOUT, FOUT], f32, kind="ExternalInput")
    t_b1 = nc.dram_tensor("b1bc", [128, FOUT], f32, kind="ExternalInput")
    t_b2 = nc.dram_tensor("b2bc", [128, FOUT], f32, kind="ExternalInput")
    t_out = nc.dram_tensor("out", [RS, FOUT], f32, kind="ExternalOutput")
    t_g2t = nc.dram_tensor("g2t", [NTAB, FIN], f16, kind="Internal")

    actf = {"gelu": mybir.ActivationFunctionType.Gelu,
            "tanh": mybir.ActivationFunctionType.Tanh}[act]

    rsems = [nc.alloc_semaphore(f"g2arr{k}") for k in range(KAG)]
    lsem = nc.alloc_semaphore("g2sent")
    dsem = nc.alloc_semaphore("g2wr")

    # SBUF state shared across the two tile contexts (raw, not pool-managed)
    g2send = nc.alloc_sbuf_tensor("g2send", [128, GP, FOUT], f16)
    stage = nc.alloc_sbuf_tensor("stage", [128, KAG, NC - 1, GPC, FOUT], f16)
    iota = nc.alloc_sbuf_tensor("iotah", [128, 128], f16)
    dinv_t = nc.alloc_sbuf_tensor("dinvt", [128, GP], f32)
    b2_t = nc.alloc_sbuf_tensor("b2t", [128, FOUT], f32)

    def run_phase(tc, L, t_idx, dst_t, coef_t, elem, src_spaces, post_fn,
                  tag, gelem=None):
        from concourse import mybir
        gelem = elem if gelem is None else gelem
        with (
            tc.tile_pool(name=f"gat{tag}", bufs=2) as gp_,
            tc.tile_pool(name=f"idx{tag}", bufs=4) as ip_,
            tc.tile_pool(name=f"agg{tag}", bufs=B, space="PSUM") as ap_,
            tc.tile_pool(name=f"post{tag}", bufs=4) as wp_,
            tc.tile_pool(name=f"sgen{tag}", bufs=8) as sp_,
            tc.tile_pool(name=f"pp{tag}", bufs=2, space="PSUM") as pp_,
        ):
            mi_by_call = {}
            for m in L["mdescs"]:
                mi_by_call.setdefault(m[0], []).append(m)
            psum_of = {}
            cur_batch = [-1]

            for ci, call in enumerate(L["calls"]):
                ntok = call["ntok"]
                ncols = ntok // 128
                bi = call["batch"]
                groups_b = L["batches"][bi]
                if bi != cur_batch[0]:
                    cur_batch[0] = bi
                    psum_of.clear()
                    for g in groups_b:
                        t = ap_.tile([128, elem], mybir.dt.float32,
                                     tag=f"a{tag}", name=f"aggp{tag}",
                                     space="PSUM")
                        psum_of[g] = t[:, :]
                gtile = gp_.tile([128, L["max_ntok"] // 128, gelem],
                                 mybir.dt.float16, tag=f"g{tag}")
                idxt = ip_.tile([128, L["max_ntok"] // 16],
                                mybir.dt.int16, tag=f"i{tag}")
                c0 = call["idx_col0"]
                nc.sync.dma_start(idxt[:, :ntok // 16],
                                  t_idx[:, c0:c0 + ntok // 16])
                # split each call across two SWDGE queues: desc-gen for the
                # two halves runs on different Q7 core pairs in parallel
                h = (ncols + 1) // 2
                tok0 = h * 128
                nc.gpsimd.dma_gather(
                    out_ap=gtile[:, :h, :],
                    in_ap=src_spaces[call["w"]],
                    idxs_ap=idxt[:, :tok0 // 16],
                    num_idxs=tok0,
                    num_idxs_reg=tok0,
                    elem_size=gelem,
                    single_packet=False,
                    queue_num=(2 * ci) % 4,
                )
                if ncols > h:
                    nc.gpsimd.dma_gather(
                        out_ap=gtile[:, h:ncols, :],
                        in_ap=src_spaces[call["w"]],
                        idxs_ap=idxt[:, tok0 // 16:ntok // 16],
                        num_idxs=ntok - tok0,
                        num_idxs_reg=ntok - tok0,
                        elem_size=gelem,
                        single_packet=False,
                        queue_num=(2 * ci + 1) % 4,
                    )
                for _, j, g, mi in mi_by_call.get(ci, []):
                    bkey = (bi, g)
                    st = L["first_call_of"][bkey] == (ci, mi)
                    sp = L["last_call_of"][bkey] == (ci, mi)
                    S = sp_.tile([128, 128], f16, tag=f"S{tag}")
                    if coef_t is not None:
                        nc.vector.tensor_scalar(
                            out=S[:], in0=iota[:, :],
                            scalar1=dst_t[:, mi:mi + 1],
                            scalar2=coef_t[:, mi:mi + 1],
                            op0=mybir.AluOpType.is_equal,
                            op1=mybir.AluOpType.mult)
                    else:
                        nc.vector.tensor_scalar(
                            out=S[:], in0=iota[:, :],
                            scalar1=dst_t[:, mi:mi + 1],
                            scalar2=None,
                            op0=mybir.AluOpType.is_equal)
                    nc.tensor.matmul(psum_of[g], lhsT=S[:],
                                     rhs=gtile[:, j, :elem],
                                     start=st, stop=sp)
                    if sp:
                        post_fn(g, psum_of[g], wp_, pp_)

    # ================= context A: layer 1 + transform + exchange sends
    with tile.TileContext(nc) as tc:
        with (
            tc.tile_pool(name="const", bufs=1) as cp,
        ):
            ident = cp.tile([128, 128], f32)
            make_identity(nc, ident[:])
            iota_i = cp.tile([128, 128], mybir.dt.int32)
            nc.gpsimd.iota(iota_i[:], pattern=[[1, 128]], base=0,
                           channel_multiplier=0)
            nc.vector.tensor_copy(iota[:, :], iota_i[:])
            w1_t = cp.tile([FIN, FOUT], f32)
            w2_t = cp.tile([FOUT, FOUT], f32)
            b1_t = cp.tile([128, FOUT], f32)
            for tt, src_t in ((w1_t, t_w1), (w2_t, t_w2), (b1_t, t_b1)):
                nc.sync.dma_start(tt[:], src_t[:, :])
            nc.sync.dma_start(dinv_t[:, :], t_dinv[:, :])
            nc.sync.dma_start(b2_t[:, :], t_b2[:, :])
            dst1_t = cp.tile([128, L1["M"]], f32)
            coef1_t = cp.tile([128, L1["M"]], f32)
            nc.sync.dma_start(dst1_t[:], t_dst1[:, :])
            nc.sync.dma_start(coef1_t[:], t_coef1[:, :])

            def send_chunk(k):
                gcnt = (lens[k] + GSZ - 1) // GSZ
                g0 = k * GPC
                src_ap = g2send[:, g0:g0 + gcnt, :]
                for i in range(1, NC):
                    rd = [None] * 8
                    rd[i] = (0, i)
                    nc.gpsimd.remote_dma_broadcast(
                        out_ap=stage[:, k, i - 1, 0:gcnt, :],
                        in_ap=src_ap,
                        remote_sem=rsems[k], local_sem=lsem,
                        rdests=rd)
                nc.gpsimd.trigger_dma(count=None)
                # own shard -> local table, overlapped with layer 1 (needs
                # no remote wait; completion counted into dsem)
                base = NC * AGL * k
                jf = lens[k] // GSZ
                rem = lens[k] - jf * GSZ
                if jf:
                    dap = t_g2t[base:base + jf * GSZ, 0:FOUT].rearrange(
                        "(j p) c -> p j c", p=GSZ)
                    nc.sync.dma_start(dap, src_ap[:, :jf, :])
                if rem:
                    nc.sync.dma_start(
                        t_g2t[base + jf * GSZ:base + lens[k], 0:FOUT],
                        src_ap[:rem, jf, :])

            def post_l1(g, agg, wp_, pp_):
                xd = wp_.tile([128, FIN], f32, tag="xd")
                nc.sync.dma_start(xd[:], t_xs[g * GSZ:(g + 1) * GSZ, :])
                v = wp_.tile([128, FIN], f32, tag="v")
                nc.vector.tensor_tensor(out=v[:], in0=xd[:], in1=agg,
                                        op=mybir.AluOpType.add)
                bank = pp_.tile([128, 512], f32, tag="pb", space="PSUM")
                vT_p = bank[:, 0:128]
                h1_p = bank[:, 128:128 + FOUT]
                z1T_p = bank[:64, 192:320]
                h2_p = bank[:, 320:320 + FOUT]
                nc.tensor.transpose(vT_p, v[:], ident[:])
                vT = wp_.tile([128, 128], f32, tag="vTs")
                nc.scalar.activation(vT[:], vT_p,
                                     mybir.ActivationFunctionType.Copy)
                nc.tensor.matmul(h1_p, lhsT=vT[:], rhs=w1_t[:], start=True,
                                 stop=True)
                h1b = wp_.tile([128, FOUT], f32, tag="h1b")
                nc.vector.tensor_tensor(out=h1b[:], in0=h1_p, in1=b1_t[:],
                                        op=mybir.AluOpType.add)
                z1 = wp_.tile([128, FOUT], f32, tag="z1")
                nc.scalar.activation(z1[:], h1b[:], actf)
                nc.tensor.transpose(z1T_p, z1[:], ident[:])
                z1T = wp_.tile([FOUT, 128], f32, tag="z1Ts")
                nc.scalar.activation(z1T[:], z1T_p,
                                     mybir.ActivationFunctionType.Copy)
                nc.tensor.matmul(h2_p, lhsT=z1T[:], rhs=w2_t[:], start=True,
                                 stop=True)
                nc.scalar.activation(g2send[:, g, :], h2_p,
                                     mybir.ActivationFunctionType.Copy,
                                     scale=dinv_t[:, g:g + 1])
                k = g // GPC
                if g == min(GP, (k + 1) * GPC) - 1:
                    send_chunk(k)

            run_phase(tc, L1, t_idx1, dst1_t, coef1_t, FIN,
                      [t_xh[w * WIN:w * WIN + winlen[w], :]
                       for w in range(NW)],
                      post_l1, "1")

    # ================= raw middle: wait for remote chunks, build g2 table
    # Peer segments only (self shards were written during layer 1). Issued
    # from the Activation engine so SP is free to prefetch layer-2 idx/const
    # tiles during this span.
    ndma = 0
    for k in range(KAG):
        nc.scalar.wait_ge(rsems[k], (NC - 1) * 2)
        gcnt = (lens[k] + GSZ - 1) // GSZ
        jf = lens[k] // GSZ          # full 128-row groups
        rem = lens[k] - jf * GSZ
        base = NC * AGL * k
        if jf == gcnt:
            dap = t_g2t[base + lens[k]:base + NC * lens[k], 0:FOUT].rearrange(
                "(s j p) c -> p (s j) c", p=GSZ, s=NC - 1).opt()
            nc.scalar.dma_start(dap, stage[:, k, :, 0:gcnt, :].opt()) \
                .then_inc(dsem, 16)
            ndma += 1
            continue
        for i in range(1, NC):
            sap = stage[:, k, i - 1, 0:gcnt, :]
            base_i = base + i * lens[k]
            if jf:
                dap = t_g2t[base_i:base_i + jf * GSZ, 0:FOUT].rearrange(
                    "(j p) c -> p j c", p=GSZ)
                nc.scalar.dma_start(dap, sap[:, :jf, :]).then_inc(dsem, 16)
                ndma += 1
            if rem:
                nc.scalar.dma_start(
                    t_g2t[base_i + jf * GSZ:base_i + lens[k], 0:FOUT],
                    sap[:rem, jf, :]).then_inc(dsem, 16)
                ndma += 1
    nc.gpsimd.wait_ge(dsem, 16 * ndma)

    # ================= context B: layer 2
    tablen = [min(WIN, NTAB - w * WIN) for w in range(NW)]
    with tile.TileContext(nc) as tc:
        with tc.tile_pool(name="constB", bufs=1) as cpb:
            dst2_t = cpb.tile([128, L2["M"]], f32)
            nc.sync.dma_start(dst2_t[:], t_dst2[:, :])

            def post_l2(g, agg, wp_, pp_):
                t1 = wp_.tile([128, FOUT], f32, tag="t1")
                nc.vector.tensor_tensor(out=t1[:], in0=agg,
                                        in1=g2send[:, g, :],
                                        op=mybir.AluOpType.add)
                t2 = wp_.tile([128, FOUT], f32, tag="t2")
                nc.scalar.activation(t2[:], t1[:],
                                     mybir.ActivationFunctionType.Copy,
                                     scale=dinv_t[:, g:g + 1])
                t3 = wp_.tile([128, FOUT], f32, tag="t3")
                nc.vector.tensor_tensor(out=t3[:], in0=t2[:], in1=b2_t[:, :],
                                        op=mybir.AluOpType.add)
                nrow = min(GSZ, RS - g * GSZ)
                nc.sync.dma_start(t_out[g * GSZ:g * GSZ + nrow, :],
                                  t3[:nrow, :])

            run_phase(tc, L2, t_idx2, dst2_t, None, FOUT,
                      [t_g2t[w * WIN:w * WIN + tablen[w], :]
                       for w in range(NW)],
                      post_l2, "2", gelem=FIN)

    nc.compile()
    return nc


def _run(inputs, act="gelu", trace=False, use_sim=False, trace_kwargs=None):
    x = np.ascontiguousarray(np.asarray(inputs["x"], np.float32))
    key = (hash(np.asarray(inputs["edge_index"]).tobytes()), act)
    if key not in _cache:
        pp = _preprocess(x, np.asarray(inputs["edge_index"]),
                         inputs["W1"], inputs["b1"], inputs["W2"],
                         inputs["b2"])
        nc = _build_nc(pp, act=act)
        _cache.clear()
        _cache[key] = (pp, nc)
    pp, nc = _cache[key]

    in_maps = _make_in_maps(pp)
    if use_sim:
        from concourse.bass_interp import MultiCoreSim
        sim = MultiCoreSim(nc, num_cores=NC, require_finite=False, require_nnan=False)
        for ci, core in sim.cores.items():
            for k, v in in_maps[ci].items():
                core.tensor(k)[:] = v
        sim.simulate()
        outs = [np.array(core.tensor("out"))
                for _, core in sorted(sim.cores.items())]
        return np.concatenate(outs, 0), None
    from concourse.bass_utils import run_bass_kernel_spmd
    res = run_bass_kernel_spmd(nc, in_maps, core_ids=list(range(NC)),
                               trace=trace, **(trace_kwargs or {}))
    out = np.concatenate([res.results[c]["out"] for c in range(NC)], 0)
    return out, res


def kernel(**inputs) -> np.ndarray:
    out, _ = _run(inputs)
    return out


def bench(inputs, act="gelu", iters=8):
    """Measure per-execution device time by chaining `iters` executions of
    the NEFF inside one jit (outputs feed the next iteration's output
    operands, defeating CSE) and comparing against a 1-iteration call."""
    import time
    import jax
    from jax.sharding import Mesh, PartitionSpec
    from jax.experimental.shard_map import shard_map
    from concourse import bass2jax as b2j

    key = (hash(np.asarray(inputs["edge_index"]).tobytes()), act)
    if key not in _cache:
        _run(inputs, act=act)   # build + correctness path
    pp, nc = _cache[key]
    b2j.install_neuronx_cc_hook()

    in_maps = _make_in_maps(pp)

    in_names, out_names, out_avals, zero_outs = [], [], [], []
    import concourse.mybir as mb
    pid_name = (nc.partition_id_tensor.name
                if nc.partition_id_tensor is not None else None)
    for alloc in nc.m.functions[0].allocations:
        if not isinstance(alloc, mb.MemoryLocationSet):
            continue
        name = alloc.memorylocations[0].name
        if alloc.kind == "ExternalInput":
            if name == pid_name:
                continue
            in_names.append(name)
        elif alloc.kind == "ExternalOutput":
            out_names.append(name)
            shape = tuple(alloc.tensor_shape)
            dtype = mb.dt.np(alloc.dtype)
            out_avals.append(jax.core.ShapedArray(shape, dtype))
            zero_outs.append(np.zeros(shape, dtype))
    n_params = len(in_names)
    all_names = in_names + out_names
    if pid_name is not None:
        all_names = all_names + [pid_name]

    def one_call(params, outs_in):
        extra = ([b2j.partition_id_tensor()] if pid_name is not None else [])
        outs = b2j._bass_exec_p.bind(
            *params, *outs_in, *extra,
            out_avals=tuple(out_avals),
            in_names=tuple(all_names),
            out_names=tuple(out_names),
            lowering_input_output_aliases=(),
            sim_require_finite=True,
            sim_require_nnan=True,
            nc=nc,
        )
        return list(outs)

    def _body(*args):
        params = list(args[:n_params])
        outs = list(args[n_params:])
        outs = one_call(params, outs)
        return tuple(outs)

    devices = jax.devices()[:NC]
    mesh = Mesh(np.asarray(devices), ("core",))
    specs = (PartitionSpec("core"),)
    per_core = [[np.asarray(m[nm]) for nm in in_names] for m in in_maps]
    concat_in = [np.concatenate([per_core[c][i] for c in range(NC)], 0)
                 for i in range(n_params)]
    concat_zeros = [np.zeros((NC * z.shape[0], *z.shape[1:]), z.dtype)
                    for z in zero_outs]

    nin = n_params + len(out_names)
    fn = jax.jit(shard_map(_body, mesh=mesh,
                           in_specs=specs * nin,
                           out_specs=specs * len(out_names),
                           check_rep=False),
                 donate_argnums=tuple(range(n_params, nin)))
    from jax.sharding import NamedSharding
    shard = NamedSharding(mesh, PartitionSpec("core"))
    dev_in = [jax.device_put(a, shard) for a in concat_in]
    outs = [jax.device_put(a, shard) for a in concat_zeros]
    outs = fn(*dev_in, *outs)          # warm: compile + first exec
    jax.block_until_ready(outs)

    results = {}
    for k in (1, iters):
        best = None
        for _ in range(3):
            t0 = time.perf_counter()
            o = outs
            for _ in range(k):
                o = fn(*dev_in, *o)
            jax.block_until_ready(o)
            dt = time.perf_counter() - t0
            outs = o
            best = dt if best is None else min(best, dt)
        results[k] = best
    per_iter_ns = (results[iters] - results[1]) / (iters - 1) * 1e9
    return per_iter_ns, results



# revision 5
# speedup vs baseline: 1.5615x; 1.1923x over previous
"""GCN layer (2x gcn_conv with GELU) on 8 Trainium2 NeuronCores — v5.

Contract: kernel(**inputs) takes the FULL inputs of reference.setup_inputs()
and returns the FULL [100000, 64] float32 output.

Strategy (graph/data parallel, sharded by destination node):
- Edges partitioned by dst across 8 cores (12500 dst nodes each).
- Per core, edges sorted by (dst-group of 128, src-window of 32768, src).
- Layer 1 "commuted": gather raw x rows (fp16) per edge via dma_gather;
  aggregate with one-hot fp16 S matrices (norm folded in) via TensorE into
  f32 PSUM, accumulated across gather calls per (batch, group).
- Dense transform (W1, GELU, W2) in f32 per 128-dst group; g2 = dinv * (z1@W2)
  converted to fp16.
- Exchange: instead of collectives, each core pushes its g2 chunks (7 chunks
  of <=2048 rows) to all 7 peers' SBUF staging via XOR-relative
  remote_dma_broadcast (1 real dest per instruction), then each receiver
  writes staging into its local DRAM g2 table (padded fp16 rows of 256 B).
- Layer 2: gather fp16 g2 rows per edge from the local table (per-core XOR
  layout), aggregate the same way, add self-loop, scale, bias, write out.

v5 vs the original v2 baseline (4.44ms -> ~2.7ms measured):
- Every dma_gather call is split into four quarters on the four SWDGE
  queues (Bacc(num_swdge_queues=4)): descriptor generation runs on four
  GpSimd Q7 core pairs in parallel and aggregation matmuls start as soon
  as their quarter lands.
- The mid-kernel stage->g2t table copies are split across the SP and ACT
  HWDGE queues per chunk instead of serializing on ACT.
- Self-loop term dinv^2*x pre-scaled on host into xs (one ACT op/group).
- g2loc (f32) dropped; layer 2 reads the f16 g2send buffer directly (one
  ACT copy/group and 25KB/partition SBUF saved).
"""
import sys
sys.path.insert(0, "/opt/trn_rl_repo")

import numpy as np

N = 100000
FIN = 128
FOUT = 64
NC = 8
RS = N // NC            # 12500 dst rows per core
GSZ = 128               # dst group size
GP = (RS + GSZ - 1) // GSZ   # 98 groups per core (last has 84 nodes)
WIN = 32768             # src index window (int16 limit)
B = 6                   # dst groups per batch (PSUM banks: B agg + 2 transform)
CAP = 8192              # max tokens per dma_gather
AGL = 2048              # g2 rows per core per exchange chunk
KAG = (RS + AGL - 1) // AGL  # 7 chunks (last 212 rows)
GPC = AGL // GSZ        # dst groups per chunk (16)

# logical NC -> real NC on TRN2 (XOR-linear involution); relative rdma dests
# are XORed in real-NC space, so peer distance d maps to BASE[d].
BASEMAP = (0, 1, 2, 3, 6, 7, 4, 5)


def _set_config(**kw):
    """Override module constants (for scaled-down tests) and derived values."""
    g = globals()
    g.update(kw)
    g["RS"] = g["N"] // g["NC"]
    g["GP"] = (g["RS"] + g["GSZ"] - 1) // g["GSZ"]
    g["KAG"] = (g["RS"] + g["AGL"] - 1) // g["AGL"]
    g["GPC"] = g["AGL"] // g["GSZ"]
    assert g["AGL"] % g["GSZ"] == 0
    _cache.clear()

_cache = {}


# ----------------------------------------------------------------- host side

def _chunk_lens():
    return [min(AGL, RS - k * AGL) for k in range(KAG)]


def _flat_g2_row(src, c):
    """Node id -> row in core c's g2 table layout.

    Table layout per chunk k: [seg 0..NC) blocks of len_k rows, where seg
    i holds the shard of the core at XOR distance i in real-NC space:
    seg(cs) = BASE[cs] ^ BASE[c].
    """
    base = np.asarray(BASEMAP, dtype=np.int64)
    cs = src // RS
    r = src % RS
    k = r // AGL
    off = r - k * AGL
    len_k = np.minimum(RS - k * AGL, AGL)
    seg = base[cs] ^ base[c]
    return NC * AGL * k + seg * len_k + off


def _build_layer(rows_by_core, gl_by_core, dl_by_core, coef_by_core):
    """Shared program structure + per-core token data for one layer."""
    NW = (N + WIN - 1) // WIN
    has_coef = coef_by_core is not None
    per_core = []
    cell_cnt = np.zeros((NC, GP, NW), dtype=np.int64)
    for c in range(NC):
        rows, gl, dl = rows_by_core[c], gl_by_core[c], dl_by_core[c]
        win = rows // WIN
        order = np.lexsort((rows, win, gl))
        rows, gl, dl, win = rows[order], gl[order], dl[order], win[order]
        coef = coef_by_core[c][order] if has_coef else None
        np.add.at(cell_cnt[c], (gl, win), 1)
        flat_sizes = cell_cnt[c].reshape(-1)
        starts = np.concatenate([[0], np.cumsum(flat_sizes)[:-1]]).reshape(GP, NW)
        per_core.append(dict(rows=rows, dl=dl.astype(np.float32), coef=coef,
                             starts=starts))

    cell_max = cell_cnt.max(axis=0)
    cell_pad = ((cell_max + 15) // 16) * 16
    empty = cell_pad.sum(axis=1) == 0
    cell_pad[empty, 0] = 16

    batches = [list(range(b, min(b + B, GP))) for b in range(0, GP, B)]
    calls = []
    mdescs = []      # [call_i, col, g, m_index]
    total_cols_idx = 0

    def close_call(bi, w, gext, pos):
        nonlocal total_cols_idx
        ntok = ((pos + 127) // 128) * 128
        if ntok == 0:
            return
        ci = len(calls)
        calls.append(dict(batch=bi, w=w, ntok=ntok, gext=dict(gext),
                          idx_col0=total_cols_idx))
        total_cols_idx += ntok // 16
        for j in range(ntok // 128):
            lo, hi = j * 128, (j + 1) * 128
            for g, (s, e) in gext.items():
                if s < hi and e > lo:
                    mdescs.append([ci, j, g, len(mdescs)])

    for bi, groups in enumerate(batches):
        for w in range(NW):
            gext = {}
            pos = 0
            for g in groups:
                sz = int(cell_pad[g, w])
                if not sz:
                    continue
                assert sz <= CAP, f"cell {g},{w} = {sz} exceeds CAP"
                if pos + sz > CAP:
                    close_call(bi, w, gext, pos)
                    gext, pos = {}, 0
                gext[g] = (pos, pos + sz)
                pos += sz
            close_call(bi, w, gext, pos)

    # first/last call (and first/last mdesc within them) per (batch, group)
    first_call_of = {}
    last_call_of = {}
    for m in mdescs:
        ci, _, g, _ = m
        bkey = (calls[ci]["batch"], g)
        if bkey not in first_call_of:
            first_call_of[bkey] = (ci, m[3])
        last_call_of[bkey] = (ci, m[3])
    M = len(mdescs)

    idx_arr = np.zeros((NC, 16, total_cols_idx), dtype=np.int16)
    dst_arr = np.full((NC, M, 128), -1.0, dtype=np.float32)
    coef_arr = np.zeros((NC, M, 128), dtype=np.float32) if has_coef else None

    for c in range(NC):
        pc = per_core[c]
        for call in calls:
            w = call["w"]
            ntok = call["ntok"]
            stream_rows = np.zeros(ntok, dtype=np.int64)
            for g, (s, e) in call["gext"].items():
                cnt = int(cell_cnt[c, g, w])
                st = pc["starts"][g, w]
                if cnt:
                    stream_rows[s:s + cnt] = pc["rows"][st:st + cnt] - w * WIN
                    stream_rows[s + cnt:e] = stream_rows[s + cnt - 1]
            c0 = call["idx_col0"]
            idx_arr[c, :, c0:c0 + ntok // 16] = (
                stream_rows.astype(np.int16).reshape(-1, 16).T)
        for ci, j, g, mi in mdescs:
            call = calls[ci]
            w = call["w"]
            s, e = call["gext"][g]
            lo, hi = j * 128, (j + 1) * 128
            a = max(s, lo)
            cnt = int(cell_cnt[c, g, w])
            st = pc["starts"][g, w]
            real_hi = min(hi, s + cnt)
            if real_hi > a:
                k0, k1 = a - s, real_hi - s
                dst_arr[c, mi, a - lo:real_hi - lo] = pc["dl"][st + k0:st + k1]
                if has_coef:
                    coef_arr[c, mi, a - lo:real_hi - lo] = \
                        pc["coef"][st + k0:st + k1]

    out = dict(calls=calls, mdescs=mdescs, M=M,
               first_call_of=first_call_of, last_call_of=last_call_of,
               idx=np.tile(idx_arr, (1, 8, 1)),
               dst=np.ascontiguousarray(dst_arr.transpose(0, 2, 1)),
               total_idx_cols=total_cols_idx,
               batches=batches,
               max_ntok=max(c_["ntok"] for c_ in calls))
    if has_coef:
        out["coef"] = np.ascontiguousarray(coef_arr.transpose(0, 2, 1))
    return out


def _preprocess(x, edge_index, W1, b1, W2, b2):
    src = np.asarray(edge_index[0], dtype=np.int64)
    dst = np.asarray(edge_index[1], dtype=np.int64)
    deg = np.bincount(dst, minlength=N).astype(np.float32) + 1.0
    dinv = (1.0 / np.sqrt(deg)).astype(np.float32)

    core = dst // RS
    l1 = dict(rows=[], gl=[], dl=[], coef=[])
    l2 = dict(rows=[], gl=[], dl=[])
    for c in range(NC):
        m = core == c
        s, d = src[m], dst[m]
        rl = d - c * RS
        l1["rows"].append(s)
        l1["gl"].append(rl // GSZ)
        l1["dl"].append(rl % GSZ)
        l1["coef"].append((dinv[s] * dinv[d]).astype(np.float32))
        l2["rows"].append(_flat_g2_row(s, c))
        l2["gl"].append(rl // GSZ)
        l2["dl"].append(rl % GSZ)

    L1 = _build_layer(l1["rows"], l1["gl"], l1["dl"], l1["coef"])
    L2 = _build_layer(l2["rows"], l2["gl"], l2["dl"], None)

    dinv_pc = np.zeros((NC, 128, GP), dtype=np.float32)
    xs_pc = np.zeros((NC, GP * GSZ, FIN), dtype=np.float32)
    for c in range(NC):
        dvp = np.zeros(GP * GSZ, dtype=np.float32)
        dvp[:RS] = dinv[c * RS:(c + 1) * RS]
        dinv_pc[c] = dvp.reshape(GP, GSZ).T
        # self-loop term pre-scaled by dinv^2 (saves one ACT op per group)
        xs_pc[c, :RS] = x[c * RS:(c + 1) * RS] *             (dinv[c * RS:(c + 1) * RS] ** 2)[:, None]

    b1bc = np.tile(np.asarray(b1, np.float32)[None, :], (128, 1))
    b2bc = np.tile(np.asarray(b2, np.float32)[None, :], (128, 1))
    return dict(L1=L1, L2=L2, dinv_pc=dinv_pc, xs_pc=xs_pc,
                b1bc=b1bc, b2bc=b2bc,
                xh=np.ascontiguousarray(np.asarray(x, np.float16)),
                W1=np.ascontiguousarray(np.asarray(W1, np.float32)),
                W2=np.ascontiguousarray(np.asarray(W2, np.float32)))


def _make_in_maps(pp, x_unused=None):
    in_maps = []
    for c in range(NC):
        in_maps.append({
            "xh": pp["xh"],
            "xs": pp["xs_pc"][c],
            "idx1": pp["L1"]["idx"][c],
            "idx2": pp["L2"]["idx"][c],
            "dst1": pp["L1"]["dst"][c],
            "coef1": pp["L1"]["coef"][c],
            "dst2": pp["L2"]["dst"][c],
            "dinv_pc": pp["dinv_pc"][c],
            "W1": pp["W1"], "W2": pp["W2"],
            "b1bc": pp["b1bc"], "b2bc": pp["b2bc"],
        })
    return in_maps


# --------------------------------------------------------------- device side

def _build_nc(pp, act="gelu"):
    import concourse.bacc as bacc
    import concourse.tile as tile
    from concourse import mybir
    from concourse.masks import make_identity

    L1, L2 = pp["L1"], pp["L2"]
    nc = bacc.Bacc(num_devices=NC, num_swdge_queues=4)
    f32 = mybir.dt.float32
    f16 = mybir.dt.float16

    NW = (N + WIN - 1) // WIN
    winlen = [min(WIN, N - w * WIN) for w in range(NW)]
    lens = _chunk_lens()
    NTAB = NC * RS  # g2 table rows (padded fp16 rows of 128)
    # chunk k of the table must sit inside one gather window
    for k in range(KAG):
        assert (NC * AGL * k) // WIN == (NC * AGL * k + NC * lens[k] - 1) // WIN

    t_xh = nc.dram_tensor("xh", [N, FIN], f16, kind="ExternalInput")
    t_xs = nc.dram_tensor("xs", [GP * GSZ, FIN], f32, kind="ExternalInput")
    t_idx1 = nc.dram_tensor("idx1", [128, L1["total_idx_cols"]],
                            mybir.dt.int16, kind="ExternalInput")
    t_idx2 = nc.dram_tensor("idx2", [128, L2["total_idx_cols"]],
                            mybir.dt.int16, kind="ExternalInput")
    t_dst1 = nc.dram_tensor("dst1", [128, L1["M"]], f32, kind="ExternalInput")
    t_coef1 = nc.dram_tensor("coef1", [128, L1["M"]], f32,
                             kind="ExternalInput")
    t_dst2 = nc.dram_tensor("dst2", [128, L2["M"]], f32, kind="ExternalInput")
    t_dinv = nc.dram_tensor("dinv_pc", [128, GP], f32, kind="ExternalInput")
    t_w1 = nc.dram_tensor("W1", [FIN, FOUT], f32, kind="ExternalInput")
    t_w2 = nc.dram_tensor("W2", [FOUT, FOUT], f32, kind="ExternalInput")
    t_b1 = nc.dram_tensor("b1bc", [128, FOUT], f32, kind="ExternalInput")
    t_b2 = nc.dram_tensor("b2bc", [128, FOUT], f32, kind="ExternalInput")
    t_out = nc.dram_tensor("out", [RS, FOUT], f32, kind="ExternalOutput")
    t_g2t = nc.dram_tensor("g2t", [NTAB, FIN], f16, kind="Internal")

    actf = {"gelu": mybir.ActivationFunctionType.Gelu,
            "tanh": mybir.ActivationFunctionType.Tanh}[act]

    rsems = [nc.alloc_semaphore(f"g2arr{k}") for k in range(KAG)]
    lsem = nc.alloc_semaphore("g2sent")
    dsem = nc.alloc_semaphore("g2wr")

    # SBUF state shared across the two tile contexts (raw, not pool-managed)
    g2send = nc.alloc_sbuf_tensor("g2send", [128, GP, FOUT], f16)
    stage = nc.alloc_sbuf_tensor("stage", [128, KAG, NC - 1, GPC, FOUT], f16)
    iota = nc.alloc_sbuf_tensor("iotah", [128, 128], f16)
    dinv_t = nc.alloc_sbuf_tensor("dinvt", [128, GP], f32)
    b2_t = nc.alloc_sbuf_tensor("b2t", [128, FOUT], f32)

    def run_phase(tc, L, t_idx, dst_t, coef_t, elem, src_spaces, post_fn,
                  tag, gelem=None):
        from concourse import mybir
        gelem = elem if gelem is None else gelem
        with (
            tc.tile_pool(name=f"gat{tag}", bufs=2) as gp_,
            tc.tile_pool(name=f"idx{tag}", bufs=4) as ip_,
            tc.tile_pool(name=f"agg{tag}", bufs=B, space="PSUM") as ap_,
            tc.tile_pool(name=f"post{tag}", bufs=4) as wp_,
            tc.tile_pool(name=f"sgen{tag}", bufs=8) as sp_,
            tc.tile_pool(name=f"pp{tag}", bufs=2, space="PSUM") as pp_,
        ):
            mi_by_call = {}
            for m in L["mdescs"]:
                mi_by_call.setdefault(m[0], []).append(m)
            psum_of = {}
            cur_batch = [-1]

            for ci, call in enumerate(L["calls"]):
                ntok = call["ntok"]
                ncols = ntok // 128
                bi = call["batch"]
                groups_b = L["batches"][bi]
                if bi != cur_batch[0]:
                    cur_batch[0] = bi
                    psum_of.clear()
                    for g in groups_b:
                        t = ap_.tile([128, elem], mybir.dt.float32,
                                     tag=f"a{tag}", name=f"aggp{tag}",
                                     space="PSUM")
                        psum_of[g] = t[:, :]
                gtile = gp_.tile([128, L["max_ntok"] // 128, gelem],
                                 mybir.dt.float16, tag=f"g{tag}")
                idxt = ip_.tile([128, L["max_ntok"] // 16],
                                mybir.dt.int16, tag=f"i{tag}")
                c0 = call["idx_col0"]
                nc.sync.dma_start(idxt[:, :ntok // 16],
                                  t_idx[:, c0:c0 + ntok // 16])
                # split each call across all four SWDGE queues: desc-gen
                # runs on four Q7 core pairs in parallel and aggregation
                # matmuls start as soon as their quarter lands
                nq = min(4, ncols)
                qs = (ncols + nq - 1) // nq
                for qi in range(nq):
                    qa, qb = qi * qs, min((qi + 1) * qs, ncols)
                    if qa >= qb:
                        break
                    nc.gpsimd.dma_gather(
                        out_ap=gtile[:, qa:qb, :],
                        in_ap=src_spaces[call["w"]],
                        idxs_ap=idxt[:, qa * 8:qb * 8],
                        num_idxs=(qb - qa) * 128,
                        num_idxs_reg=(qb - qa) * 128,
                        elem_size=gelem,
                        single_packet=False,
                        queue_num=(4 * ci + qi) % 4,
                    )
                for _, j, g, mi in mi_by_call.get(ci, []):
                    bkey = (bi, g)
                    st = L["first_call_of"][bkey] == (ci, mi)
                    sp = L["last_call_of"][bkey] == (ci, mi)
                    S = sp_.tile([128, 128], f16, tag=f"S{tag}")
                    if coef_t is not None:
                        nc.vector.tensor_scalar(
                            out=S[:], in0=iota[:, :],
                            scalar1=dst_t[:, mi:mi + 1],
                            scalar2=coef_t[:, mi:mi + 1],
                            op0=mybir.AluOpType.is_equal,
                            op1=mybir.AluOpType.mult)
                    else:
                        nc.vector.tensor_scalar(
                            out=S[:], in0=iota[:, :],
                            scalar1=dst_t[:, mi:mi + 1],
                            scalar2=None,
                            op0=mybir.AluOpType.is_equal)
                    nc.tensor.matmul(psum_of[g], lhsT=S[:],
                                     rhs=gtile[:, j, :elem],
                                     start=st, stop=sp)
                    if sp:
                        post_fn(g, psum_of[g], wp_, pp_)

    # ================= context A: layer 1 + transform + exchange sends
    with tile.TileContext(nc) as tc:
        with (
            tc.tile_pool(name="const", bufs=1) as cp,
        ):
            ident = cp.tile([128, 128], f32)
            make_identity(nc, ident[:])
            iota_i = cp.tile([128, 128], mybir.dt.int32)
            nc.gpsimd.iota(iota_i[:], pattern=[[1, 128]], base=0,
                           channel_multiplier=0)
            nc.vector.tensor_copy(iota[:, :], iota_i[:])
            w1_t = cp.tile([FIN, FOUT], f32)
            w2_t = cp.tile([FOUT, FOUT], f32)
            b1_t = cp.tile([128, FOUT], f32)
            for tt, src_t in ((w1_t, t_w1), (w2_t, t_w2), (b1_t, t_b1)):
                nc.sync.dma_start(tt[:], src_t[:, :])
            nc.sync.dma_start(dinv_t[:, :], t_dinv[:, :])
            nc.sync.dma_start(b2_t[:, :], t_b2[:, :])
            dst1_t = cp.tile([128, L1["M"]], f32)
            coef1_t = cp.tile([128, L1["M"]], f32)
            nc.sync.dma_start(dst1_t[:], t_dst1[:, :])
            nc.sync.dma_start(coef1_t[:], t_coef1[:, :])

            def send_chunk(k):
                gcnt = (lens[k] + GSZ - 1) // GSZ
                g0 = k * GPC
                src_ap = g2send[:, g0:g0 + gcnt, :]
                for i in range(1, NC):
                    rd = [None] * 8
                    rd[i] = (0, i)
                    nc.gpsimd.remote_dma_broadcast(
                        out_ap=stage[:, k, i - 1, 0:gcnt, :],
                        in_ap=src_ap,
                        remote_sem=rsems[k], local_sem=lsem,
                        rdests=rd)
                nc.gpsimd.trigger_dma(count=None)
                # own shard -> local table, overlapped with layer 1 (needs
                # no remote wait; completion counted into dsem)
                base = NC * AGL * k
                jf = lens[k] // GSZ
                rem = lens[k] - jf * GSZ
                if jf:
                    dap = t_g2t[base:base + jf * GSZ, 0:FOUT].rearrange(
                        "(j p) c -> p j c", p=GSZ)
                    nc.sync.dma_start(dap, src_ap[:, :jf, :])
                if rem:
                    nc.sync.dma_start(
                        t_g2t[base + jf * GSZ:base + lens[k], 0:FOUT],
                        src_ap[:rem, jf, :])

            def post_l1(g, agg, wp_, pp_):
                xd = wp_.tile([128, FIN], f32, tag="xd")
                nc.sync.dma_start(xd[:], t_xs[g * GSZ:(g + 1) * GSZ, :])
                v = wp_.tile([128, FIN], f32, tag="v")
                nc.vector.tensor_tensor(out=v[:], in0=xd[:], in1=agg,
                                        op=mybir.AluOpType.add)
                bank = pp_.tile([128, 512], f32, tag="pb", space="PSUM")
                vT_p = bank[:, 0:128]
                h1_p = bank[:, 128:128 + FOUT]
                z1T_p = bank[:64, 192:320]
                h2_p = bank[:, 320:320 + FOUT]
                nc.tensor.transpose(vT_p, v[:], ident[:])
                vT = wp_.tile([128, 128], f32, tag="vTs")
                nc.scalar.activation(vT[:], vT_p,
                                     mybir.ActivationFunctionType.Copy)
                nc.tensor.matmul(h1_p, lhsT=vT[:], rhs=w1_t[:], start=True,
                                 stop=True)
                h1b = wp_.tile([128, FOUT], f32, tag="h1b")
                nc.vector.tensor_tensor(out=h1b[:], in0=h1_p, in1=b1_t[:],
                                        op=mybir.AluOpType.add)
                z1 = wp_.tile([128, FOUT], f32, tag="z1")
                nc.scalar.activation(z1[:], h1b[:], actf)
                nc.tensor.transpose(z1T_p, z1[:], ident[:])
                z1T = wp_.tile([FOUT, 128], f32, tag="z1Ts")
                nc.scalar.activation(z1T[:], z1T_p,
                                     mybir.ActivationFunctionType.Copy)
                nc.tensor.matmul(h2_p, lhsT=z1T[:], rhs=w2_t[:], start=True,
                                 stop=True)
                nc.scalar.activation(g2send[:, g, :], h2_p,
                                     mybir.ActivationFunctionType.Copy,
                                     scale=dinv_t[:, g:g + 1])
                k = g // GPC
                if g == min(GP, (k + 1) * GPC) - 1:
                    send_chunk(k)

            run_phase(tc, L1, t_idx1, dst1_t, coef1_t, FIN,
                      [t_xh[w * WIN:w * WIN + winlen[w], :]
                       for w in range(NW)],
                      post_l1, "1")

    # ================= raw middle: wait for remote chunks, build g2 table
    # Peer segments only (self shards were written during layer 1). Issued
    # from the Activation engine so SP is free to prefetch layer-2 idx/const
    # tiles during this span.
    ndma = 0
    engs = [nc.sync, nc.scalar]
    for k in range(KAG):
        for e in engs:
            e.wait_ge(rsems[k], (NC - 1) * 2)
        gcnt = (lens[k] + GSZ - 1) // GSZ
        jf = lens[k] // GSZ          # full 128-row groups
        rem = lens[k] - jf * GSZ
        base = NC * AGL * k
        if jf == gcnt:
            # split the 7 peer segments across the 3 engine queues so the
            # copies run concurrently instead of serializing on ACT
            for ei, (s0, s1) in enumerate(((0, 4), (4, 7))):
                dap = t_g2t[base + (1 + s0) * lens[k]:
                            base + (1 + s1) * lens[k], 0:FOUT].rearrange(
                    "(s j p) c -> p (s j) c", p=GSZ, s=s1 - s0).opt()
                engs[ei].dma_start(
                    dap, stage[:, k, s0:s1, 0:gcnt, :].opt()) \
                    .then_inc(dsem, 16)
                ndma += 1
            continue
        for i in range(1, NC):
            sap = stage[:, k, i - 1, 0:gcnt, :]
            base_i = base + i * lens[k]
            e = engs[i % 2]
            if jf:
                dap = t_g2t[base_i:base_i + jf * GSZ, 0:FOUT].rearrange(
                    "(j p) c -> p j c", p=GSZ)
                e.dma_start(dap, sap[:, :jf, :]).then_inc(dsem, 16)
                ndma += 1
            if rem:
                e.dma_start(
                    t_g2t[base_i + jf * GSZ:base_i + lens[k], 0:FOUT],
                    sap[:rem, jf, :]).then_inc(dsem, 16)
                ndma += 1
    nc.gpsimd.wait_ge(dsem, 16 * ndma)

    # ================= context B: layer 2
    tablen = [min(WIN, NTAB - w * WIN) for w in range(NW)]
    with tile.TileContext(nc) as tc:
        with tc.tile_pool(name="constB", bufs=1) as cpb:
            dst2_t = cpb.tile([128, L2["M"]], f32)
            nc.sync.dma_start(dst2_t[:], t_dst2[:, :])

            def post_l2(g, agg, wp_, pp_):
                t1 = wp_.tile([128, FOUT], f32, tag="t1")
                nc.vector.tensor_tensor(out=t1[:], in0=agg,
                                        in1=g2send[:, g, :],
                                        op=mybir.AluOpType.add)
                t2 = wp_.tile([128, FOUT], f32, tag="t2")
                nc.scalar.activation(t2[:], t1[:],
                                     mybir.ActivationFunctionType.Copy,
                                     scale=dinv_t[:, g:g + 1])
                t3 = wp_.tile([128, FOUT], f32, tag="t3")
                nc.vector.tensor_tensor(out=t3[:], in0=t2[:], in1=b2_t[:, :],
                                        op=mybir.AluOpType.add)
                nrow = min(GSZ, RS - g * GSZ)
                nc.sync.dma_start(t_out[g * GSZ:g * GSZ + nrow, :],
                                  t3[:nrow, :])

            run_phase(tc, L2, t_idx2, dst2_t, None, FOUT,
                      [t_g2t[w * WIN:w * WIN + tablen[w], :]
                       for w in range(NW)],
                      post_l2, "2", gelem=FIN)

    nc.compile()
    return nc


def _run(inputs, act="gelu", trace=False, use_sim=False, trace_kwargs=None):
    x = np.ascontiguousarray(np.asarray(inputs["x"], np.float32))
    key = (hash(np.asarray(inputs["edge_index"]).tobytes()), act)
    if key not in _cache:
        pp = _preprocess(x, np.asarray(inputs["edge_index"]),
                         inputs["W1"], inputs["b1"], inputs["W2"],
                         inputs["b2"])
        nc = _build_nc(pp, act=act)
        _cache.clear()
        _cache[key] = (pp, nc)
    pp, nc = _cache[key]

    in_maps = _make_in_maps(pp)
    if use_sim:
        from concourse.bass_interp import MultiCoreSim
        sim = MultiCoreSim(nc, num_cores=NC, require_finite=False, require_nnan=False)
        for ci, core in sim.cores.items():
            for k, v in in_maps[ci].items():
                core.tensor(k)[:] = v
        sim.simulate()
        outs = [np.array(core.tensor("out"))
                for _, core in sorted(sim.cores.items())]
        return np.concatenate(outs, 0), None
    from concourse.bass_utils import run_bass_kernel_spmd
    res = run_bass_kernel_spmd(nc, in_maps, core_ids=list(range(NC)),
                               trace=trace, **(trace_kwargs or {}))
    out = np.concatenate([res.results[c]["out"] for c in range(NC)], 0)
    return out, res


def kernel(**inputs) -> np.ndarray:
    out, _ = _run(inputs)
    return out


def bench(inputs, act="gelu", iters=8):
    """Measure per-execution device time by chaining `iters` executions of
    the NEFF inside one jit (outputs feed the next iteration's output
    operands, defeating CSE) and comparing against a 1-iteration call."""
    import time
    import jax
    from jax.sharding import Mesh, PartitionSpec
    from jax.experimental.shard_map import shard_map
    from concourse import bass2jax as b2j

    key = (hash(np.asarray(inputs["edge_index"]).tobytes()), act)
    if key not in _cache:
        _run(inputs, act=act)   # build + correctness path
    pp, nc = _cache[key]
    b2j.install_neuronx_cc_hook()

    in_maps = _make_in_maps(pp)

    in_names, out_names, out_avals, zero_outs = [], [], [], []
    import concourse.mybir as mb
    pid_name = (nc.partition_id_tensor.name
                if nc.partition_id_tensor is not None else None)
    for alloc in nc.m.functions[0].allocations:
        if not isinstance(alloc, mb.MemoryLocationSet):
            continue
        name = alloc.memorylocations[0].name
        if alloc.kind == "ExternalInput":
            if name == pid_name:
                continue
            in_names.append(name)
        elif alloc.kind == "ExternalOutput":
            out_names.append(name)
            shape = tuple(alloc.tensor_shape)
            dtype = mb.dt.np(alloc.dtype)
            out_avals.append(jax.core.ShapedArray(shape, dtype))
            zero_outs.append(np.zeros(shape, dtype))
    n_params = len(in_names)
    all_names = in_names + out_names
    if pid_name is not None:
        all_names = all_names + [pid_name]

    def one_call(params, outs_in):
        extra = ([b2j.partition_id_tensor()] if pid_name is not None else [])
        outs = b2j._bass_exec_p.bind(
            *params, *outs_in, *extra,
            out_avals=tuple(out_avals),
            in_names=tuple(all_names),
            out_names=tuple(out_names),
            lowering_input_output_aliases=(),
            sim_require_finite=True,
            sim_require_nnan=True,
            nc=nc,
        )
        return list(outs)

    def _body(*args):
        params = list(args[:n_params])
        outs = list(args[n_params:])
        outs = one_call(params, outs)
        return tuple(outs)

    devices = jax.devices()[:NC]
    mesh = Mesh(np.asarray(devices), ("core",))
    specs = (PartitionSpec("core"),)
    per_core = [[np.asarray(m[nm]) for nm in in_names] for m in in_maps]
    concat_in = [np.concatenate([per_core[c][i] for c in range(NC)], 0)
                 for i in range(n_params)]
    concat_zeros = [np.zeros((NC * z.shape[0], *z.shape[1:]), z.dtype)
                    for z in zero_outs]

    nin = n_params + len(out_names)
    fn = jax.jit(shard_map(_body, mesh=mesh,
                           in_specs=specs * nin,
                           out_specs=specs * len(out_names),
                           check_rep=False),
                 donate_argnums=tuple(range(n_params, nin)))
    from jax.sharding import NamedSharding
    shard = NamedSharding(mesh, PartitionSpec("core"))
    dev_in = [jax.device_put(a, shard) for a in concat_in]
    outs = [jax.device_put(a, shard) for a in concat_zeros]
    outs = fn(*dev_in, *outs)          # warm: compile + first exec
    jax.block_until_ready(outs)

    results = {}
    for k in (1, iters):
        best = None
        for _ in range(3):
            t0 = time.perf_counter()
            o = outs
            for _ in range(k):
                o = fn(*dev_in, *o)
            jax.block_until_ready(o)
            dt = time.perf_counter() - t0
            outs = o
            best = dt if best is None else min(best, dt)
        results[k] = best
    per_iter_ns = (results[iters] - results[1]) / (iters - 1) * 1e9
    return per_iter_ns, results



# revision 6
# speedup vs baseline: 1.5886x; 1.0174x over previous
"""GCN layer (2x gcn_conv with GELU) on 8 Trainium2 NeuronCores — v6.

Contract: kernel(**inputs) takes the FULL inputs of reference.setup_inputs()
and returns the FULL [100000, 64] float32 output.

Strategy (graph/data parallel, sharded by destination node):
- Edges partitioned by dst across 8 cores (12500 dst nodes each).
- Per core, edges sorted by (dst-group of 128, src-window of 32768, src).
- Layer 1 "commuted": gather raw x rows (fp16) per edge via dma_gather;
  aggregate with one-hot fp16 S matrices (norm folded in) via TensorE into
  f32 PSUM, accumulated across gather calls per (batch, group).
- Dense transform (W1, GELU, W2) in f32 per 128-dst group; g2 = dinv * (z1@W2)
  converted to fp16.
- Exchange: instead of collectives, each core pushes its g2 chunks (7 chunks
  of <=2048 rows) to all 7 peers' SBUF staging via XOR-relative
  remote_dma_broadcast (1 real dest per instruction), then each receiver
  writes staging into its local DRAM g2 table (padded fp16 rows of 256 B).
- Layer 2: gather fp16 g2 rows per edge from the local table (per-core XOR
  layout), aggregate the same way, add self-loop, scale, bias, write out.

v6 vs the original v2 baseline (4.44ms -> ~2.2ms paired-bench):
- Every dma_gather call is split into four quarters on the four SWDGE
  queues (Bacc(num_swdge_queues=4)): descriptor generation runs on four
  GpSimd Q7 core pairs in parallel and aggregation matmuls start as soon
  as their quarter lands. NOTE: Tile locks DMA-SW sem lane L to queue L%4;
  the quarter order satisfies this by construction — keep it aligned.
- Gather pipeline triple-buffered (gat bufs=3, idx bufs=6): call N+2's
  gathers overlap call N's aggregation.
- Mid-kernel stage->g2t table copies split across the SP and ACT HWDGE
  queues per chunk instead of serializing on ACT.
- Self-loop term dinv^2*x pre-scaled on host into xs (one ACT op/group).
- g2loc (f32) dropped; layer 2 reads the f16 g2send buffer directly (one
  ACT copy/group and 25KB/partition SBUF saved).
"""
import sys
sys.path.insert(0, "/opt/trn_rl_repo")

import numpy as np

N = 100000
FIN = 128
FOUT = 64
NC = 8
RS = N // NC            # 12500 dst rows per core
GSZ = 128               # dst group size
GP = (RS + GSZ - 1) // GSZ   # 98 groups per core (last has 84 nodes)
WIN = 32768             # src index window (int16 limit)
B = 6                   # dst groups per batch (PSUM banks: B agg + 2 transform)
CAP = 8192              # max tokens per dma_gather
AGL = 2048              # g2 rows per core per exchange chunk
KAG = (RS + AGL - 1) // AGL  # 7 chunks (last 212 rows)
GPC = AGL // GSZ        # dst groups per chunk (16)

# logical NC -> real NC on TRN2 (XOR-linear involution); relative rdma dests
# are XORed in real-NC space, so peer distance d maps to BASE[d].
BASEMAP = (0, 1, 2, 3, 6, 7, 4, 5)


def _set_config(**kw):
    """Override module constants (for scaled-down tests) and derived values."""
    g = globals()
    g.update(kw)
    g["RS"] = g["N"] // g["NC"]
    g["GP"] = (g["RS"] + g["GSZ"] - 1) // g["GSZ"]
    g["KAG"] = (g["RS"] + g["AGL"] - 1) // g["AGL"]
    g["GPC"] = g["AGL"] // g["GSZ"]
    assert g["AGL"] % g["GSZ"] == 0
    _cache.clear()

_cache = {}


# ----------------------------------------------------------------- host side

def _chunk_lens():
    return [min(AGL, RS - k * AGL) for k in range(KAG)]


def _flat_g2_row(src, c):
    """Node id -> row in core c's g2 table layout.

    Table layout per chunk k: [seg 0..NC) blocks of len_k rows, where seg
    i holds the shard of the core at XOR distance i in real-NC space:
    seg(cs) = BASE[cs] ^ BASE[c].
    """
    base = np.asarray(BASEMAP, dtype=np.int64)
    cs = src // RS
    r = src % RS
    k = r // AGL
    off = r - k * AGL
    len_k = np.minimum(RS - k * AGL, AGL)
    seg = base[cs] ^ base[c]
    return NC * AGL * k + seg * len_k + off


def _build_layer(rows_by_core, gl_by_core, dl_by_core, coef_by_core):
    """Shared program structure + per-core token data for one layer."""
    NW = (N + WIN - 1) // WIN
    has_coef = coef_by_core is not None
    per_core = []
    cell_cnt = np.zeros((NC, GP, NW), dtype=np.int64)
    for c in range(NC):
        rows, gl, dl = rows_by_core[c], gl_by_core[c], dl_by_core[c]
        win = rows // WIN
        order = np.lexsort((rows, win, gl))
        rows, gl, dl, win = rows[order], gl[order], dl[order], win[order]
        coef = coef_by_core[c][order] if has_coef else None
        np.add.at(cell_cnt[c], (gl, win), 1)
        flat_sizes = cell_cnt[c].reshape(-1)
        starts = np.concatenate([[0], np.cumsum(flat_sizes)[:-1]]).reshape(GP, NW)
        per_core.append(dict(rows=rows, dl=dl.astype(np.float32), coef=coef,
                             starts=starts))

    cell_max = cell_cnt.max(axis=0)
    cell_pad = ((cell_max + 15) // 16) * 16
    empty = cell_pad.sum(axis=1) == 0
    cell_pad[empty, 0] = 16

    batches = [list(range(b, min(b + B, GP))) for b in range(0, GP, B)]
    calls = []
    mdescs = []      # [call_i, col, g, m_index]
    total_cols_idx = 0

    def close_call(bi, w, gext, pos):
        nonlocal total_cols_idx
        ntok = ((pos + 127) // 128) * 128
        if ntok == 0:
            return
        ci = len(calls)
        calls.append(dict(batch=bi, w=w, ntok=ntok, gext=dict(gext),
                          idx_col0=total_cols_idx))
        total_cols_idx += ntok // 16
        for j in range(ntok // 128):
            lo, hi = j * 128, (j + 1) * 128
            for g, (s, e) in gext.items():
                if s < hi and e > lo:
                    mdescs.append([ci, j, g, len(mdescs)])

    for bi, groups in enumerate(batches):
        for w in range(NW):
            gext = {}
            pos = 0
            for g in groups:
                sz = int(cell_pad[g, w])
                if not sz:
                    continue
                assert sz <= CAP, f"cell {g},{w} = {sz} exceeds CAP"
                if pos + sz > CAP:
                    close_call(bi, w, gext, pos)
                    gext, pos = {}, 0
                gext[g] = (pos, pos + sz)
                pos += sz
            close_call(bi, w, gext, pos)

    # first/last call (and first/last mdesc within them) per (batch, group)
    first_call_of = {}
    last_call_of = {}
    for m in mdescs:
        ci, _, g, _ = m
        bkey = (calls[ci]["batch"], g)
        if bkey not in first_call_of:
            first_call_of[bkey] = (ci, m[3])
        last_call_of[bkey] = (ci, m[3])
    M = len(mdescs)

    idx_arr = np.zeros((NC, 16, total_cols_idx), dtype=np.int16)
    dst_arr = np.full((NC, M, 128), -1.0, dtype=np.float32)
    coef_arr = np.zeros((NC, M, 128), dtype=np.float32) if has_coef else None

    for c in range(NC):
        pc = per_core[c]
        for call in calls:
            w = call["w"]
            ntok = call["ntok"]
            stream_rows = np.zeros(ntok, dtype=np.int64)
            for g, (s, e) in call["gext"].items():
                cnt = int(cell_cnt[c, g, w])
                st = pc["starts"][g, w]
                if cnt:
                    stream_rows[s:s + cnt] = pc["rows"][st:st + cnt] - w * WIN
                    stream_rows[s + cnt:e] = stream_rows[s + cnt - 1]
            c0 = call["idx_col0"]
            idx_arr[c, :, c0:c0 + ntok // 16] = (
                stream_rows.astype(np.int16).reshape(-1, 16).T)
        for ci, j, g, mi in mdescs:
            call = calls[ci]
            w = call["w"]
            s, e = call["gext"][g]
            lo, hi = j * 128, (j + 1) * 128
            a = max(s, lo)
            cnt = int(cell_cnt[c, g, w])
            st = pc["starts"][g, w]
            real_hi = min(hi, s + cnt)
            if real_hi > a:
                k0, k1 = a - s, real_hi - s
                dst_arr[c, mi, a - lo:real_hi - lo] = pc["dl"][st + k0:st + k1]
                if has_coef:
                    coef_arr[c, mi, a - lo:real_hi - lo] = \
                        pc["coef"][st + k0:st + k1]

    out = dict(calls=calls, mdescs=mdescs, M=M,
               first_call_of=first_call_of, last_call_of=last_call_of,
               idx=np.tile(idx_arr, (1, 8, 1)),
               dst=np.ascontiguousarray(dst_arr.transpose(0, 2, 1)),
               total_idx_cols=total_cols_idx,
               batches=batches,
               max_ntok=max(c_["ntok"] for c_ in calls))
    if has_coef:
        out["coef"] = np.ascontiguousarray(coef_arr.transpose(0, 2, 1))
    return out


def _preprocess(x, edge_index, W1, b1, W2, b2):
    src = np.asarray(edge_index[0], dtype=np.int64)
    dst = np.asarray(edge_index[1], dtype=np.int64)
    deg = np.bincount(dst, minlength=N).astype(np.float32) + 1.0
    dinv = (1.0 / np.sqrt(deg)).astype(np.float32)

    core = dst // RS
    l1 = dict(rows=[], gl=[], dl=[], coef=[])
    l2 = dict(rows=[], gl=[], dl=[])
    for c in range(NC):
        m = core == c
        s, d = src[m], dst[m]
        rl = d - c * RS
        l1["rows"].append(s)
        l1["gl"].append(rl // GSZ)
        l1["dl"].append(rl % GSZ)
        l1["coef"].append((dinv[s] * dinv[d]).astype(np.float32))
        l2["rows"].append(_flat_g2_row(s, c))
        l2["gl"].append(rl // GSZ)
        l2["dl"].append(rl % GSZ)

    L1 = _build_layer(l1["rows"], l1["gl"], l1["dl"], l1["coef"])
    L2 = _build_layer(l2["rows"], l2["gl"], l2["dl"], None)

    dinv_pc = np.zeros((NC, 128, GP), dtype=np.float32)
    xs_pc = np.zeros((NC, GP * GSZ, FIN), dtype=np.float32)
    for c in range(NC):
        dvp = np.zeros(GP * GSZ, dtype=np.float32)
        dvp[:RS] = dinv[c * RS:(c + 1) * RS]
        dinv_pc[c] = dvp.reshape(GP, GSZ).T
        # self-loop term pre-scaled by dinv^2 (saves one ACT op per group)
        xs_pc[c, :RS] = x[c * RS:(c + 1) * RS] *             (dinv[c * RS:(c + 1) * RS] ** 2)[:, None]

    b1bc = np.tile(np.asarray(b1, np.float32)[None, :], (128, 1))
    b2bc = np.tile(np.asarray(b2, np.float32)[None, :], (128, 1))
    return dict(L1=L1, L2=L2, dinv_pc=dinv_pc, xs_pc=xs_pc,
                b1bc=b1bc, b2bc=b2bc,
                xh=np.ascontiguousarray(np.asarray(x, np.float16)),
                W1=np.ascontiguousarray(np.asarray(W1, np.float32)),
                W2=np.ascontiguousarray(np.asarray(W2, np.float32)))


def _make_in_maps(pp, x_unused=None):
    in_maps = []
    for c in range(NC):
        in_maps.append({
            "xh": pp["xh"],
            "xs": pp["xs_pc"][c],
            "idx1": pp["L1"]["idx"][c],
            "idx2": pp["L2"]["idx"][c],
            "dst1": pp["L1"]["dst"][c],
            "coef1": pp["L1"]["coef"][c],
            "dst2": pp["L2"]["dst"][c],
            "dinv_pc": pp["dinv_pc"][c],
            "W1": pp["W1"], "W2": pp["W2"],
            "b1bc": pp["b1bc"], "b2bc": pp["b2bc"],
        })
    return in_maps


# --------------------------------------------------------------- device side

def _build_nc(pp, act="gelu"):
    import concourse.bacc as bacc
    import concourse.tile as tile
    from concourse import mybir
    from concourse.masks import make_identity

    L1, L2 = pp["L1"], pp["L2"]
    nc = bacc.Bacc(num_devices=NC, num_swdge_queues=4)
    f32 = mybir.dt.float32
    f16 = mybir.dt.float16

    NW = (N + WIN - 1) // WIN
    winlen = [min(WIN, N - w * WIN) for w in range(NW)]
    lens = _chunk_lens()
    NTAB = NC * RS  # g2 table rows (padded fp16 rows of 128)
    # chunk k of the table must sit inside one gather window
    for k in range(KAG):
        assert (NC * AGL * k) // WIN == (NC * AGL * k + NC * lens[k] - 1) // WIN

    t_xh = nc.dram_tensor("xh", [N, FIN], f16, kind="ExternalInput")
    t_xs = nc.dram_tensor("xs", [GP * GSZ, FIN], f32, kind="ExternalInput")
    t_idx1 = nc.dram_tensor("idx1", [128, L1["total_idx_cols"]],
                            mybir.dt.int16, kind="ExternalInput")
    t_idx2 = nc.dram_tensor("idx2", [128, L2["total_idx_cols"]],
                            mybir.dt.int16, kind="ExternalInput")
    t_dst1 = nc.dram_tensor("dst1", [128, L1["M"]], f32, kind="ExternalInput")
    t_coef1 = nc.dram_tensor("coef1", [128, L1["M"]], f32,
                             kind="ExternalInput")
    t_dst2 = nc.dram_tensor("dst2", [128, L2["M"]], f32, kind="ExternalInput")
    t_dinv = nc.dram_tensor("dinv_pc", [128, GP], f32, kind="ExternalInput")
    t_w1 = nc.dram_tensor("W1", [FIN, FOUT], f32, kind="ExternalInput")
    t_w2 = nc.dram_tensor("W2", [FOUT, FOUT], f32, kind="ExternalInput")
    t_b1 = nc.dram_tensor("b1bc", [128, FOUT], f32, kind="ExternalInput")
    t_b2 = nc.dram_tensor("b2bc", [128, FOUT], f32, kind="ExternalInput")
    t_out = nc.dram_tensor("out", [RS, FOUT], f32, kind="ExternalOutput")
    t_g2t = nc.dram_tensor("g2t", [NTAB, FIN], f16, kind="Internal")

    actf = {"gelu": mybir.ActivationFunctionType.Gelu,
            "tanh": mybir.ActivationFunctionType.Tanh}[act]

    rsems = [nc.alloc_semaphore(f"g2arr{k}") for k in range(KAG)]
    lsem = nc.alloc_semaphore("g2sent")
    dsem = nc.alloc_semaphore("g2wr")

    # SBUF state shared across the two tile contexts (raw, not pool-managed)
    g2send = nc.alloc_sbuf_tensor("g2send", [128, GP, FOUT], f16)
    stage = nc.alloc_sbuf_tensor("stage", [128, KAG, NC - 1, GPC, FOUT], f16)
    iota = nc.alloc_sbuf_tensor("iotah", [128, 128], f16)
    dinv_t = nc.alloc_sbuf_tensor("dinvt", [128, GP], f32)
    b2_t = nc.alloc_sbuf_tensor("b2t", [128, FOUT], f32)

    def run_phase(tc, L, t_idx, dst_t, coef_t, elem, src_spaces, post_fn,
                  tag, gelem=None):
        from concourse import mybir
        gelem = elem if gelem is None else gelem
        with (
            tc.tile_pool(name=f"gat{tag}", bufs=3) as gp_,
            tc.tile_pool(name=f"idx{tag}", bufs=6) as ip_,
            tc.tile_pool(name=f"agg{tag}", bufs=B, space="PSUM") as ap_,
            tc.tile_pool(name=f"post{tag}", bufs=4) as wp_,
            tc.tile_pool(name=f"sgen{tag}", bufs=8) as sp_,
            tc.tile_pool(name=f"pp{tag}", bufs=2, space="PSUM") as pp_,
        ):
            mi_by_call = {}
            for m in L["mdescs"]:
                mi_by_call.setdefault(m[0], []).append(m)
            psum_of = {}
            cur_batch = [-1]

            for ci, call in enumerate(L["calls"]):
                ntok = call["ntok"]
                ncols = ntok // 128
                bi = call["batch"]
                groups_b = L["batches"][bi]
                if bi != cur_batch[0]:
                    cur_batch[0] = bi
                    psum_of.clear()
                    for g in groups_b:
                        t = ap_.tile([128, elem], mybir.dt.float32,
                                     tag=f"a{tag}", name=f"aggp{tag}",
                                     space="PSUM")
                        psum_of[g] = t[:, :]
                gtile = gp_.tile([128, L["max_ntok"] // 128, gelem],
                                 mybir.dt.float16, tag=f"g{tag}")
                idxt = ip_.tile([128, L["max_ntok"] // 16],
                                mybir.dt.int16, tag=f"i{tag}")
                c0 = call["idx_col0"]
                nc.sync.dma_start(idxt[:, :ntok // 16],
                                  t_idx[:, c0:c0 + ntok // 16])
                # split each call across all four SWDGE queues: desc-gen
                # runs on four Q7 core pairs in parallel and aggregation
                # matmuls start as soon as their quarter lands
                nq = min(4, ncols)
                qs = (ncols + nq - 1) // nq
                for qi in range(nq):
                    qa, qb = qi * qs, min((qi + 1) * qs, ncols)
                    if qa >= qb:
                        break
                    nc.gpsimd.dma_gather(
                        out_ap=gtile[:, qa:qb, :],
                        in_ap=src_spaces[call["w"]],
                        idxs_ap=idxt[:, qa * 8:qb * 8],
                        num_idxs=(qb - qa) * 128,
                        num_idxs_reg=(qb - qa) * 128,
                        elem_size=gelem,
                        single_packet=False,
                        queue_num=(4 * ci + qi) % 4,
                    )
                for _, j, g, mi in mi_by_call.get(ci, []):
                    bkey = (bi, g)
                    st = L["first_call_of"][bkey] == (ci, mi)
                    sp = L["last_call_of"][bkey] == (ci, mi)
                    S = sp_.tile([128, 128], f16, tag=f"S{tag}")
                    if coef_t is not None:
                        nc.vector.tensor_scalar(
                            out=S[:], in0=iota[:, :],
                            scalar1=dst_t[:, mi:mi + 1],
                            scalar2=coef_t[:, mi:mi + 1],
                            op0=mybir.AluOpType.is_equal,
                            op1=mybir.AluOpType.mult)
                    else:
                        nc.vector.tensor_scalar(
                            out=S[:], in0=iota[:, :],
                            scalar1=dst_t[:, mi:mi + 1],
                            scalar2=None,
                            op0=mybir.AluOpType.is_equal)
                    nc.tensor.matmul(psum_of[g], lhsT=S[:],
                                     rhs=gtile[:, j, :elem],
                                     start=st, stop=sp)
                    if sp:
                        post_fn(g, psum_of[g], wp_, pp_)

    # ================= context A: layer 1 + transform + exchange sends
    with tile.TileContext(nc) as tc:
        with (
            tc.tile_pool(name="const", bufs=1) as cp,
        ):
            ident = cp.tile([128, 128], f32)
            make_identity(nc, ident[:])
            iota_i = cp.tile([128, 128], mybir.dt.int32)
            nc.gpsimd.iota(iota_i[:], pattern=[[1, 128]], base=0,
                           channel_multiplier=0)
            nc.vector.tensor_copy(iota[:, :], iota_i[:])
            w1_t = cp.tile([FIN, FOUT], f32)
            w2_t = cp.tile([FOUT, FOUT], f32)
            b1_t = cp.tile([128, FOUT], f32)
            for tt, src_t in ((w1_t, t_w1), (w2_t, t_w2), (b1_t, t_b1)):
                nc.sync.dma_start(tt[:], src_t[:, :])
            nc.sync.dma_start(dinv_t[:, :], t_dinv[:, :])
            nc.sync.dma_start(b2_t[:, :], t_b2[:, :])
            dst1_t = cp.tile([128, L1["M"]], f32)
            coef1_t = cp.tile([128, L1["M"]], f32)
            nc.sync.dma_start(dst1_t[:], t_dst1[:, :])
            nc.sync.dma_start(coef1_t[:], t_coef1[:, :])

            def send_chunk(k):
                gcnt = (lens[k] + GSZ - 1) // GSZ
                g0 = k * GPC
                src_ap = g2send[:, g0:g0 + gcnt, :]
                for i in range(1, NC):
                    rd = [None] * 8
                    rd[i] = (0, i)
                    nc.gpsimd.remote_dma_broadcast(
                        out_ap=stage[:, k, i - 1, 0:gcnt, :],
                        in_ap=src_ap,
                        remote_sem=rsems[k], local_sem=lsem,
                        rdests=rd)
                nc.gpsimd.trigger_dma(count=None)
                # own shard -> local table, overlapped with layer 1 (needs
                # no remote wait; completion counted into dsem)
                base = NC * AGL * k
                jf = lens[k] // GSZ
                rem = lens[k] - jf * GSZ
                if jf:
                    dap = t_g2t[base:base + jf * GSZ, 0:FOUT].rearrange(
                        "(j p) c -> p j c", p=GSZ)
                    nc.sync.dma_start(dap, src_ap[:, :jf, :])
                if rem:
                    nc.sync.dma_start(
                        t_g2t[base + jf * GSZ:base + lens[k], 0:FOUT],
                        src_ap[:rem, jf, :])

            def post_l1(g, agg, wp_, pp_):
                xd = wp_.tile([128, FIN], f32, tag="xd")
                nc.sync.dma_start(xd[:], t_xs[g * GSZ:(g + 1) * GSZ, :])
                v = wp_.tile([128, FIN], f32, tag="v")
                nc.vector.tensor_tensor(out=v[:], in0=xd[:], in1=agg,
                                        op=mybir.AluOpType.add)
                bank = pp_.tile([128, 512], f32, tag="pb", space="PSUM")
                vT_p = bank[:, 0:128]
                h1_p = bank[:, 128:128 + FOUT]
                z1T_p = bank[:64, 192:320]
                h2_p = bank[:, 320:320 + FOUT]
                nc.tensor.transpose(vT_p, v[:], ident[:])
                vT = wp_.tile([128, 128], f32, tag="vTs")
                nc.scalar.activation(vT[:], vT_p,
                                     mybir.ActivationFunctionType.Copy)
                nc.tensor.matmul(h1_p, lhsT=vT[:], rhs=w1_t[:], start=True,
                                 stop=True)
                h1b = wp_.tile([128, FOUT], f32, tag="h1b")
                nc.vector.tensor_tensor(out=h1b[:], in0=h1_p, in1=b1_t[:],
                                        op=mybir.AluOpType.add)
                z1 = wp_.tile([128, FOUT], f32, tag="z1")
                nc.scalar.activation(z1[:], h1b[:], actf)
                nc.tensor.transpose(z1T_p, z1[:], ident[:])
                z1T = wp_.tile([FOUT, 128], f32, tag="z1Ts")
                nc.scalar.activation(z1T[:], z1T_p,
                                     mybir.ActivationFunctionType.Copy)
                nc.tensor.matmul(h2_p, lhsT=z1T[:], rhs=w2_t[:], start=True,
                                 stop=True)
                nc.scalar.activation(g2send[:, g, :], h2_p,
                                     mybir.ActivationFunctionType.Copy,
                                     scale=dinv_t[:, g:g + 1])
                k = g // GPC
                if g == min(GP, (k + 1) * GPC) - 1:
                    send_chunk(k)

            run_phase(tc, L1, t_idx1, dst1_t, coef1_t, FIN,
                      [t_xh[w * WIN:w * WIN + winlen[w], :]
                       for w in range(NW)],
                      post_l1, "1")

    # ================= raw middle: wait for remote chunks, build g2 table
    # Peer segments only (self shards were written during layer 1). Issued
    # from the Activation engine so SP is free to prefetch layer-2 idx/const
    # tiles during this span.
    ndma = 0
    engs = [nc.sync, nc.scalar]
    for k in range(KAG):
        for e in engs:
            e.wait_ge(rsems[k], (NC - 1) * 2)
        gcnt = (lens[k] + GSZ - 1) // GSZ
        jf = lens[k] // GSZ          # full 128-row groups
        rem = lens[k] - jf * GSZ
        base = NC * AGL * k
        if jf == gcnt:
            # split the 7 peer segments across the 3 engine queues so the
            # copies run concurrently instead of serializing on ACT
            for ei, (s0, s1) in enumerate(((0, 4), (4, 7))):
                dap = t_g2t[base + (1 + s0) * lens[k]:
                            base + (1 + s1) * lens[k], 0:FOUT].rearrange(
                    "(s j p) c -> p (s j) c", p=GSZ, s=s1 - s0).opt()
                engs[ei].dma_start(
                    dap, stage[:, k, s0:s1, 0:gcnt, :].opt()) \
                    .then_inc(dsem, 16)
                ndma += 1
            continue
        for i in range(1, NC):
            sap = stage[:, k, i - 1, 0:gcnt, :]
            base_i = base + i * lens[k]
            e = engs[i % 2]
            if jf:
                dap = t_g2t[base_i:base_i + jf * GSZ, 0:FOUT].rearrange(
                    "(j p) c -> p j c", p=GSZ)
                e.dma_start(dap, sap[:, :jf, :]).then_inc(dsem, 16)
                ndma += 1
            if rem:
                e.dma_start(
                    t_g2t[base_i + jf * GSZ:base_i + lens[k], 0:FOUT],
                    sap[:rem, jf, :]).then_inc(dsem, 16)
                ndma += 1
    nc.gpsimd.wait_ge(dsem, 16 * ndma)

    # ================= context B: layer 2
    tablen = [min(WIN, NTAB - w * WIN) for w in range(NW)]
    with tile.TileContext(nc) as tc:
        with tc.tile_pool(name="constB", bufs=1) as cpb:
            dst2_t = cpb.tile([128, L2["M"]], f32)
            nc.sync.dma_start(dst2_t[:], t_dst2[:, :])

            def post_l2(g, agg, wp_, pp_):
                t1 = wp_.tile([128, FOUT], f32, tag="t1")
                nc.vector.tensor_tensor(out=t1[:], in0=agg,
                                        in1=g2send[:, g, :],
                                        op=mybir.AluOpType.add)
                t2 = wp_.tile([128, FOUT], f32, tag="t2")
                nc.scalar.activation(t2[:], t1[:],
                                     mybir.ActivationFunctionType.Copy,
                                     scale=dinv_t[:, g:g + 1])
                t3 = wp_.tile([128, FOUT], f32, tag="t3")
                nc.vector.tensor_tensor(out=t3[:], in0=t2[:], in1=b2_t[:, :],
                                        op=mybir.AluOpType.add)
                nrow = min(GSZ, RS - g * GSZ)
                nc.sync.dma_start(t_out[g * GSZ:g * GSZ + nrow, :],
                                  t3[:nrow, :])

            run_phase(tc, L2, t_idx2, dst2_t, None, FOUT,
                      [t_g2t[w * WIN:w * WIN + tablen[w], :]
                       for w in range(NW)],
                      post_l2, "2", gelem=FIN)

    nc.compile()
    return nc


def _run(inputs, act="gelu", trace=False, use_sim=False, trace_kwargs=None):
    x = np.ascontiguousarray(np.asarray(inputs["x"], np.float32))
    key = (hash(np.asarray(inputs["edge_index"]).tobytes()), act)
    if key not in _cache:
        pp = _preprocess(x, np.asarray(inputs["edge_index"]),
                         inputs["W1"], inputs["b1"], inputs["W2"],
                         inputs["b2"])
        nc = _build_nc(pp, act=act)
        _cache.clear()
        _cache[key] = (pp, nc)
    pp, nc = _cache[key]

    in_maps = _make_in_maps(pp)
    if use_sim:
        from concourse.bass_interp import MultiCoreSim
        sim = MultiCoreSim(nc, num_cores=NC, require_finite=False, require_nnan=False)
        for ci, core in sim.cores.items():
            for k, v in in_maps[ci].items():
                core.tensor(k)[:] = v
        sim.simulate()
        outs = [np.array(core.tensor("out"))
                for _, core in sorted(sim.cores.items())]
        return np.concatenate(outs, 0), None
    from concourse.bass_utils import run_bass_kernel_spmd
    res = run_bass_kernel_spmd(nc, in_maps, core_ids=list(range(NC)),
                               trace=trace, **(trace_kwargs or {}))
    out = np.concatenate([res.results[c]["out"] for c in range(NC)], 0)
    return out, res


def kernel(**inputs) -> np.ndarray:
    out, _ = _run(inputs)
    return out


def bench(inputs, act="gelu", iters=8):
    """Measure per-execution device time by chaining `iters` executions of
    the NEFF inside one jit (outputs feed the next iteration's output
    operands, defeating CSE) and comparing against a 1-iteration call."""
    import time
    import jax
    from jax.sharding import Mesh, PartitionSpec
    from jax.experimental.shard_map import shard_map
    from concourse import bass2jax as b2j

    key = (hash(np.asarray(inputs["edge_index"]).tobytes()), act)
    if key not in _cache:
        _run(inputs, act=act)   # build + correctness path
    pp, nc = _cache[key]
    b2j.install_neuronx_cc_hook()

    in_maps = _make_in_maps(pp)

    in_names, out_names, out_avals, zero_outs = [], [], [], []
    import concourse.mybir as mb
    pid_name = (nc.partition_id_tensor.name
                if nc.partition_id_tensor is not None else None)
    for alloc in nc.m.functions[0].allocations:
        if not isinstance(alloc, mb.MemoryLocationSet):
            continue
        name = alloc.memorylocations[0].name
        if alloc.kind == "ExternalInput":
            if name == pid_name:
                continue
            in_names.append(name)
        elif alloc.kind == "ExternalOutput":
            out_names.append(name)
            shape = tuple(alloc.tensor_shape)
            dtype = mb.dt.np(alloc.dtype)
            out_avals.append(jax.core.ShapedArray(shape, dtype))
            zero_outs.append(np.zeros(shape, dtype))
    n_params = len(in_names)
    all_names = in_names + out_names
    if pid_name is not None:
        all_names = all_names + [pid_name]

    def one_call(params, outs_in):
        extra = ([b2j.partition_id_tensor()] if pid_name is not None else [])
        outs = b2j._bass_exec_p.bind(
            *params, *outs_in, *extra,
            out_avals=tuple(out_avals),
            in_names=tuple(all_names),
            out_names=tuple(out_names),
            lowering_input_output_aliases=(),
            sim_require_finite=True,
            sim_require_nnan=True,
            nc=nc,
        )
        return list(outs)

    def _body(*args):
        params = list(args[:n_params])
        outs = list(args[n_params:])
        outs = one_call(params, outs)
        return tuple(outs)

    devices = jax.devices()[:NC]
    mesh = Mesh(np.asarray(devices), ("core",))
    specs = (PartitionSpec("core"),)
    per_core = [[np.asarray(m[nm]) for nm in in_names] for m in in_maps]
    concat_in = [np.concatenate([per_core[c][i] for c in range(NC)], 0)
                 for i in range(n_params)]
    concat_zeros = [np.zeros((NC * z.shape[0], *z.shape[1:]), z.dtype)
                    for z in zero_outs]

    nin = n_params + len(out_names)
    fn = jax.jit(shard_map(_body, mesh=mesh,
                           in_specs=specs * nin,
                           out_specs=specs * len(out_names),
                           check_rep=False),
                 donate_argnums=tuple(range(n_params, nin)))
    from jax.sharding import NamedSharding
    shard = NamedSharding(mesh, PartitionSpec("core"))
    dev_in = [jax.device_put(a, shard) for a in concat_in]
    outs = [jax.device_put(a, shard) for a in concat_zeros]
    outs = fn(*dev_in, *outs)          # warm: compile + first exec
    jax.block_until_ready(outs)

    results = {}
    for k in (1, iters):
        best = None
        for _ in range(3):
            t0 = time.perf_counter()
            o = outs
            for _ in range(k):
                o = fn(*dev_in, *o)
            jax.block_until_ready(o)
            dt = time.perf_counter() - t0
            outs = o
            best = dt if best is None else min(best, dt)
        results[k] = best
    per_iter_ns = (results[iters] - results[1]) / (iters - 1) * 1e9
    return per_iter_ns, results

